# revision 7
# baseline (speedup 1.0000x reference)
"""DynamicSincConv1d Trainium2 kernel (v2).

Data-parallel over batch: 8 batch elements -> 8 NeuronCores. All heavy
math runs on-device in fp16 matmuls (1 cyc/row on the PE) with fp32 PSUM
accumulation; elementwise work is fp16 on the DVE where possible.

Math notes (on top of the v1 symmetry tricks):
 - STFT/irFFT are matmuls against host-baked DFT matrices with windows
   folded in; the sinc bank is symmetric about k=128 so only d=|k-128|
   in [0,128) is materialized and every filter spectrum is real.
 - z = ta_d*wid + pi*eps is produced directly by the broadcast matmul:
   the one-hot selector is pre-scaled by ta_d and an appended ones-row
   carries the +pi*eps bias.
 - sinc(z) = P(z^2) (degree-3 minimax fit, |err| < 1e-6 over the full
   z range |z| <= pi/2) evaluated together with the amp multiply in a
   single custom DVE op: out = (u*(c1 + u*(c2 + u*c3)))*amp + amp.
 - The 4 sinc banks are pre-summed on the DVE so the filter DFT is a
   single 128x128 matmul stream per (oi, a/b) tile.
 - Overlap-add is folded into the iSTFT matmul accumulation: the four
   frame-shifted contributions accumulate in PSUM via column-shifted
   moving operands, then one env-normalize + bias pass finishes y.
 - F is padded 1025 -> 1026; the junk frame never reaches the output
   because the fused-OLA matmuls exclude it by construction.
"""

import math
import numpy as np

B, CI, I, O, S = 8, 32, 2, 4, 4
K, HOP, T = 256, 64, 65536
F = T // HOP + 1             # 1025 real frames
FE = F + 1                   # padded frame count (col 1025 is junk)
H2 = (T + K) // HOP + 2      # 1030 padded hop columns (frame f uses cols f..f+3)
PI = math.pi
PIEPS = PI * 1e-6
# degree-3 minimax fit of sinc(z) = 1 + u*(c1 + u*(c2 + u*c3)), u = z^2
SC1, SC2, SC3 = -0.16665935405036883, 0.008315297713095644, -0.00018570764930803712
M1S = 16.0                   # m1 scale (fp16 range); folded out via invt
ICS = 16.0                   # ic scale; folded out via invt
CH = [(0, 512), (512, 512)]  # main column chunks; tail cols [1024:1026]

_prog_cache = {}


def _register_sinc_amp():
    import concourse.dve_ops as dve_ops
    from concourse.dve_spec import Spec, Src0, Src1, C0, C1, C2, sq, lower
    from concourse.dve_uop import DveOpSpec

    for op in dve_ops.OPS:
        if op.name == "SINC_AMP_ANT":
            return op
    u = sq(Src0)
    body = (u * (C0 + u * (C1 + u * C2))) * Src1 + Src1

    def ref(in0, in1, s0, s1, imm2):
        uu = np.square(in0.astype(np.float32))
        return ((uu * (s0 + uu * (s1 + uu * imm2))) * in1 + in1).astype(np.float32)

    spec = Spec(body=body, reference=ref)
    row = max(dve_ops._SUB_OPCODE_FOR_NAME.values()) + 1
    assert row < 0x20
    shas = {}
    for ver in ("v3", "v4"):
        uops = lower(spec, ver=ver)
        shas[ver] = DveOpSpec(
            name="SINC_AMP_ANT", opcode=row, uops=uops, rd1_en=True
        ).sha(ver)
    op = dve_ops.DveOp("SINC_AMP_ANT", spec, subdim=False, uops_sha=shas)
    dve_ops.OPS.append(op)
    dve_ops.CUSTOM_DVE_SPECS["SINC_AMP_ANT"] = spec
    dve_ops._SUB_OPCODE_FOR_NAME["SINC_AMP_ANT"] = row
    return op


def _consts():
    n = np.arange(K, dtype=np.float64)
    ola = 0.5 * (1.0 - np.cos(2.0 * np.pi * n / K))
    fir = 0.42 - 0.5 * np.cos(2.0 * np.pi * n / K) + 0.08 * np.cos(4.0 * np.pi * n / K)

    d = np.arange(128, dtype=np.float64)
    ta = (PI * d / K)                                  # pi*d/256

    # M1[d, fb] = (-1)^fb * c_d * fir[128+d]/(S*K) * cos(2*pi*d*fb/K), x M1S
    fb = np.arange(K // 2 + 1, dtype=np.float64)       # 0..128
    cd = np.where(d == 0, 1.0, 2.0)
    m1 = (((-1.0) ** fb)[None, :] * cd[:, None] * fir[128 + d.astype(int)][:, None]
          / (S * K) * np.cos(2.0 * np.pi * np.outer(d, fb) / K)) * M1S
    m1a = np.ascontiguousarray(m1[:, 0:128]).astype(np.float16)
    m1b = np.concatenate([m1[:, 128:129], m1[:, 1:128]], axis=1).astype(np.float16)

    # STFT weights: wx[j][r, col]; k = 64 j + r; fbpack col layout
    kk = np.arange(K, dtype=np.float64)
    ang = 2.0 * np.pi * np.outer(kk, fb) / K           # [256, 129]
    wre = ola[:, None] * np.cos(ang)
    wim = -ola[:, None] * np.sin(ang)
    colsA = wre[:, 0:128]
    colsB = np.concatenate([wre[:, 128:129], wim[:, 1:128]], axis=1)
    wx_full = np.concatenate([colsA, colsB], axis=1)   # [256, 256]
    wx = wx_full.reshape(4, 64, 256)
    wx_h = np.ascontiguousarray(
        wx.transpose(1, 0, 2).reshape(64, 1024)).astype(np.float16)

    # iSTFT: IC[fbpack_row, n] with ola folded, x ICS
    cp = np.where(fb == 0, 1.0, 2.0)
    icre = (cp[:, None] / K) * np.cos(2.0 * np.pi * np.outer(fb, n) / K) * ola[None, :] * ICS
    icim = (-2.0 / K) * np.sin(2.0 * np.pi * np.outer(fb, n) / K) * ola[None, :] * ICS
    ica = np.ascontiguousarray(icre[0:128]).astype(np.float16)                    # [128, 256]
    icb = np.concatenate([icre[128:129], icim[1:128]], axis=0).astype(np.float16)  # [128, 256]

    # env inverse, arranged [r, p]; absorb 1/(M1S*ICS)
    ola2 = ola * ola
    env_q = np.zeros((1028, 64), dtype=np.float64)
    for j in range(4):
        env_q[j:F + j, :] += ola2[64 * j:64 * j + 64][None, :]
    invt = (1.0 / (env_q[2:2 + 1024, :] * (M1S * ICS))).T.astype(np.float32)
    invt = np.ascontiguousarray(invt)                  # [64, 1024]

    # z-broadcast selector [33, 4096]: rows q<32: (q==ois)*ta_d; row 32: pieps
    ta16 = ta.astype(np.float16)
    e2z = np.zeros((33, 32 * 128), dtype=np.float16)
    e2a = np.zeros((32, 32 * 128), dtype=np.float16)
    for q in range(32):
        e2z[q, q * 128:(q + 1) * 128] = ta16
        e2a[q, q * 128:(q + 1) * 128] = 1.0
    e2z[32, :] = np.float16(PIEPS)
    ones_row = np.ones((1, FE), dtype=np.float16)
    return dict(m1a=m1a, m1b=m1b, wx_h=wx_h, ica=ica, icb=icb, invt=invt,
                e2z=e2z, e2a=e2a, ones_row=ones_row)


def _build_program():
    import concourse.bacc as bacc
    import concourse.mybir as mybir
    import concourse.tile as tile

    sinc_op = _register_sinc_amp()

    f16 = mybir.dt.float16
    f32 = mybir.dt.float32
    AF = mybir.ActivationFunctionType
    ALU = mybir.AluOpType

    nc = bacc.Bacc("TRN2", target_bir_lowering=False, debug=False, num_devices=8)

    d_in = nc.dram_tensor("d_in", [16, 128, H2], f16, kind="ExternalInput")
    xd_in = nc.dram_tensor("xd_in", [64, 2 * H2], f16, kind="ExternalInput")
    w1t_in = nc.dram_tensor("w1t_in", [128, 2048], f16, kind="ExternalInput")
    w2t_in = nc.dram_tensor("w2t_in", [32, 64], f16, kind="ExternalInput")
    b1_in = nc.dram_tensor("b1_in", [32, 1], f32, kind="ExternalInput")
    b2a_in = nc.dram_tensor("b2a_in", [32, 1], f32, kind="ExternalInput")
    b2w_in = nc.dram_tensor("b2w_in", [32, 1], f32, kind="ExternalInput")
    e2z_in = nc.dram_tensor("e2z_in", [33, 4096], f16, kind="ExternalInput")
    e2a_in = nc.dram_tensor("e2a_in", [32, 4096], f16, kind="ExternalInput")
    m1a_in = nc.dram_tensor("m1a_in", [128, 128], f16, kind="ExternalInput")
    m1b_in = nc.dram_tensor("m1b_in", [128, 128], f16, kind="ExternalInput")
    wx_in = nc.dram_tensor("wx_in", [64, 1024], f16, kind="ExternalInput")
    ica_in = nc.dram_tensor("ica_in", [128, 256], f16, kind="ExternalInput")
    icb_in = nc.dram_tensor("icb_in", [128, 256], f16, kind="ExternalInput")
    invt_in = nc.dram_tensor("invt_in", [64, 1024], f32, kind="ExternalInput")
    bias_in = nc.dram_tensor("bias_in", [64, 4], f32, kind="ExternalInput")
    ones_in = nc.dram_tensor("ones_in", [1, FE], f16, kind="ExternalInput")
    yt_out = nc.dram_tensor("yt_out", [64, 4096], f32, kind="ExternalOutput")

    with tile.TileContext(nc) as tc:
        with tc.tile_pool(name="cpool", bufs=1) as cpool:
            w1t_sb = cpool.tile([128, 2048], f16, tag="w1t")
            w2t_sb = cpool.tile([32, 64], f16, tag="w2t")
            b1_sb = cpool.tile([32, 1], f32, tag="b1")
            b2a_sb = cpool.tile([32, 1], f32, tag="b2a")
            b2w_sb = cpool.tile([32, 1], f32, tag="b2w")
            e2z_sb = cpool.tile([33, 4096], f16, tag="e2z")
            e2a_sb = cpool.tile([32, 4096], f16, tag="e2a")
            m1a_sb = cpool.tile([128, 128], f16, tag="m1a")
            m1b_sb = cpool.tile([128, 128], f16, tag="m1b")
            wx_sb = cpool.tile([64, 1024], f16, tag="wx")
            ica_sb = cpool.tile([128, 256], f16, tag="ica")
            icb_sb = cpool.tile([128, 256], f16, tag="icb")
            invt_sb = cpool.tile([64, 1024], f32, tag="invt")
            bias_sb = cpool.tile([64, 4], f32, tag="bias")
            xd_sb = cpool.tile([64, 2 * H2], f16, tag="xd")
            h_sb = cpool.tile([32, FE], f16, tag="h")
            wa_sb = cpool.tile([33, 2 * FE], f16, tag="wa")
            xa_sb = cpool.tile([128, 2 * FE], f16, tag="xa")
            xb_sb = cpool.tile([128, 2 * FE], f16, tag="xb")
            yt_sb = cpool.tile([64, 4096], f32, tag="yt")

            for t_sb, t_in in ((w1t_sb, w1t_in), (w2t_sb, w2t_in), (b1_sb, b1_in),
                               (b2a_sb, b2a_in), (b2w_sb, b2w_in),
                               (e2z_sb, e2z_in), (e2a_sb, e2a_in),
                               (m1a_sb, m1a_in), (m1b_sb, m1b_in),
                               (wx_sb, wx_in), (ica_sb, ica_in),
                               (icb_sb, icb_in), (invt_sb, invt_in),
                               (bias_sb, bias_in), (xd_sb, xd_in)):
                nc.sync.dma_start(t_sb[:], t_in[:])
            # ones row of wa (carries the +pi*eps bias through the z matmul)
            nc.sync.dma_start(wa_sb[32:33, 0:FE], ones_in[:])

            # ---- stage 1: conditioning conv -> h [32, FE] fp16 ----
            with tc.tile_pool(name="dpool", bufs=1) as dpool, \
                 tc.tile_pool(name="ps1", bufs=2, space="PSUM") as ps1:
                dts = []
                for c in range(16):
                    dtile = dpool.tile([128, H2], f16, tag=f"d{c}")
                    nc.sync.dma_start(dtile[:], d_in[c])
                    dts.append(dtile)
                for (f0, nf) in CH + [(1024, 2)]:
                    ps = ps1.tile([32, nf], f32, tag="ps1")
                    idx = 0
                    for c in range(16):
                        for j in range(4):
                            nc.tensor.matmul(
                                ps[:],
                                w1t_sb[:, (c * 4 + j) * 32:(c * 4 + j + 1) * 32],
                                dts[c][:, f0 + j:f0 + j + nf],
                                start=(idx == 0), stop=(idx == 63))
                            idx += 1
                    nc.scalar.activation(h_sb[:, f0:f0 + nf], ps[:], AF.Identity,
                                         bias=b1_sb[:, 0:1])
                lt = dpool.tile([32, FE], f16, tag="lt")
                nc.vector.tensor_scalar(lt[:], h_sb[:], 0.01, None, ALU.mult)
                nc.vector.tensor_max(h_sb[:], h_sb[:], lt[:])

            # ---- stage 2: 1x1 conv + tanh -> wa (wid at [0:FE], amp at [FE:2FE]) ----
            with tc.tile_pool(name="ps2", bufs=2, space="PSUM") as ps2:
                for (f0, nf) in CH + [(1024, 2)]:
                    pw = ps2.tile([32, nf], f32, tag="ps2w")
                    nc.tensor.matmul(pw[:], w2t_sb[:, 32:64], h_sb[:, f0:f0 + nf],
                                     start=True, stop=True)
                    nc.scalar.activation(wa_sb[0:32, f0:f0 + nf], pw[:], AF.Tanh,
                                         bias=b2w_sb[:, 0:1])
                    pa = ps2.tile([32, nf], f32, tag="ps2a")
                    nc.tensor.matmul(pa[:], w2t_sb[:, 0:32], h_sb[:, f0:f0 + nf],
                                     start=True, stop=True)
                    nc.scalar.activation(wa_sb[0:32, FE + f0:FE + f0 + nf], pa[:],
                                         AF.Tanh, bias=b2a_sb[:, 0:1])

            # ---- stage 4: STFT of x -> xa/xb [128, (i,f)] fp16 ----
            with tc.tile_pool(name="ps4", bufs=2, space="PSUM") as ps4:
                for i in range(2):
                    for (mt, xdst) in ((0, xa_sb), (1, xb_sb)):
                        for (f0, nf) in CH + [(1024, 2)]:
                            ps = ps4.tile([128, nf], f32, tag="ps4")
                            for j in range(4):
                                nc.tensor.matmul(
                                    ps[:],
                                    wx_sb[:, j * 256 + mt * 128: j * 256 + mt * 128 + 128],
                                    xd_sb[:, i * H2 + f0 + j:i * H2 + f0 + j + nf],
                                    start=(j == 0), stop=(j == 3))
                            nc.scalar.activation(xdst[:, i * FE + f0:i * FE + f0 + nf],
                                                 ps[:], AF.Copy)

            # ---- per half: sinc synth + DFT + cmul + fused iSTFT/OLA ----
            for half in range(2):
                with tc.tile_pool(name="fp", bufs=1) as fppool:
                    # fpab layout: [m(2)][oil(4)][FE] filter spectra, fp16
                    fpab = fppool.tile([128, 8 * FE], f16, tag="fpab")
                    with tc.tile_pool(name="ftp", bufs=2) as ftpool, \
                         tc.tile_pool(name="fsp", bufs=2) as fspool, \
                         tc.tile_pool(name="a16p", bufs=2) as a16p, \
                         tc.tile_pool(name="zps", bufs=1, space="PSUM") as zps, \
                         tc.tile_pool(name="aps", bufs=1, space="PSUM") as aps, \
                         tc.tile_pool(name="tps", bufs=1, space="PSUM") as tps, \
                         tc.tile_pool(name="dps", bufs=1, space="PSUM") as dps:
                        for oil in range(4):
                            oi = half * 4 + oil
                            ftoi = ftpool.tile([128, 4 * FE], f16, tag="ftoi")
                            # z/amp tails for all 4 s: z at [0:8], amp at [8:16]
                            tl = tps.tile([128, 16], f32, tag="tl")
                            a16t = a16p.tile([128, 8], f16, tag="a16t")
                            for s in range(4):
                                ois = oi * 4 + s
                                esl = slice(ois * 128, (ois + 1) * 128)
                                nc.tensor.matmul(
                                    tl[:, s * 2:s * 2 + 2],
                                    e2z_sb[:, esl], wa_sb[0:33, 1024:1026],
                                    start=True, stop=True)
                                nc.tensor.matmul(
                                    tl[:, 8 + s * 2:8 + s * 2 + 2],
                                    e2a_sb[:, esl], wa_sb[0:32, FE + 1024:FE + 1026],
                                    start=True, stop=True)
                                ap = aps.tile([128, 1024], f32, tag="ap")
                                a16 = a16p.tile([128, 1024], f16, tag="a16")
                                zp = zps.tile([128, 1024], f32, tag="zp")
                                for (f0, nf) in CH:
                                    nc.tensor.matmul(
                                        ap[:, f0:f0 + nf],
                                        e2a_sb[:, esl],
                                        wa_sb[0:32, FE + f0:FE + f0 + nf],
                                        start=True, stop=True)
                                    nc.tensor.matmul(
                                        zp[:, f0:f0 + nf],
                                        e2z_sb[:, esl], wa_sb[0:33, f0:f0 + nf],
                                        start=True, stop=True)
                                # amp psum -> sbuf fp16 (alternate Act/Pool)
                                nc.scalar.activation(a16[:], ap[:], AF.Copy)
                                nc.vector._custom_dve(
                                    sinc_op,
                                    out=ftoi[:, s * FE:s * FE + 1024],
                                    in0=zp[:], in1=a16[:],
                                    s0=SC1, s1=SC2, imm2=SC3)
                            nc.scalar.activation(a16t[:], tl[:, 8:16], AF.Copy)
                            # batched tail sinc for all 4 s of this oi
                            ft_r = ftoi[:].rearrange("p (s f) -> p s f", s=4)
                            nc.vector._custom_dve(
                                sinc_op,
                                out=ft_r[:, :, 1024:1026],
                                in0=tl[:, 0:8], in1=a16t[:],
                                s0=SC1, s1=SC2, imm2=SC3)

                            # ---- s-presum: ftsum = sum_s ft_s ----
                            fs1 = fspool.tile([128, FE], f16, tag="fs1")
                            fsum = fspool.tile([128, FE], f16, tag="fsum")
                            nc.vector.tensor_add(fs1[:], ftoi[:, 0:FE],
                                                 ftoi[:, FE:2 * FE])
                            nc.vector.tensor_add(fsum[:], ftoi[:, 2 * FE:3 * FE],
                                                 ftoi[:, 3 * FE:4 * FE])
                            nc.vector.tensor_add(fsum[:], fsum[:], fs1[:])

                            # ---- DFT: R_m = m1_m.T @ ftsum ----
                            dtl = tps.tile([128, 4], f32, tag="dtl")
                            for (mi, m1sb) in ((0, m1a_sb), (1, m1b_sb)):
                                dp = dps.tile([128, 1024], f32, tag="dp")
                                for (f0, nf) in CH:
                                    nc.tensor.matmul(dp[:, f0:f0 + nf], m1sb[:],
                                                     fsum[:, f0:f0 + nf],
                                                     start=True, stop=True)
                                nc.tensor.matmul(dtl[:, mi * 2:mi * 2 + 2], m1sb[:],
                                                 fsum[:, 1024:1026],
                                                 start=True, stop=True)
                                nc.scalar.activation(
                                    fpab[:, (mi * 4 + oil) * FE:(mi * 4 + oil) * FE + 1024],
                                    dp[:], AF.Copy)
                            fp_r = fpab[:].rearrange("p (m f) -> p m f", m=2)
                            nc.scalar.activation(
                                fp_r[:, :, oil * FE + 1024:oil * FE + 1026],
                                dtl[:].rearrange("p (m t) -> p m t", m=2), AF.Copy)

                    # ---- stage 5 + fused iSTFT/OLA + env/bias ----
                    with tc.tile_pool(name="yp5", bufs=2) as ypool, \
                         tc.tile_pool(name="tt5", bufs=2) as ttpool, \
                         tc.tile_pool(name="ops", bufs=2, space="PSUM") as ops, \
                         tc.tile_pool(name="etp", bufs=2) as etpool:
                        for ol in range(2):
                            o = half * 2 + ol
                            ya = ypool.tile([128, FE], f16, tag="ya")
                            yb = ypool.tile([128, FE], f16, tag="yb")
                            ta_t = ttpool.tile([128, 2 * FE], f16, tag="tat")
                            nc.vector.tensor_mul(
                                ta_t[:], xa_sb[:],
                                fpab[:, (ol * 2) * FE:(ol * 2 + 2) * FE])
                            nc.vector.tensor_add(ya[:], ta_t[:, 0:FE],
                                                 ta_t[:, FE:2 * FE])
                            tb_t = ttpool.tile([128, 2 * FE], f16, tag="tbt")
                            nc.vector.tensor_mul(
                                tb_t[:], xb_sb[:],
                                fpab[:, (4 + ol * 2) * FE:(4 + ol * 2 + 2) * FE])
                            nc.vector.tensor_add(yb[:], tb_t[:, 0:FE],
                                                 tb_t[:, FE:2 * FE])

                            for c, m0 in ((0, 0), (1, 512)):
                                yp = ops.tile([64, 512], f32, tag="yp")
                                # pick a j whose column window is full-width to
                                # carry start=True; trimmed edges accumulate.
                                jorder = (0, 1, 2, 3) if c == 0 else (1, 0, 2, 3)
                                first = True
                                n_mm = 8
                                k = 0
                                for j in jorder:
                                    lo = m0 + 2 - j          # ya col of out col 0
                                    o0, o1 = 0, 512          # out col range
                                    if lo < 0:
                                        o0 = -lo
                                    if lo + 512 > F:
                                        o1 = F - lo
                                    for ic_sb, ysrc in ((ica_sb, ya), (icb_sb, yb)):
                                        k += 1
                                        nc.tensor.matmul(
                                            yp[:, o0:o1],
                                            ic_sb[:, j * 64:(j + 1) * 64],
                                            ysrc[:, lo + o0:lo + o1],
                                            start=first, stop=(k == n_mm),
                                            skip_group_check=True)
                                        first = False
                                et = etpool.tile([64, 512], f32, tag="et")
                                nc.vector.tensor_mul(et[:], yp[:],
                                                     invt_sb[:, m0:m0 + 512])
                                nc.gpsimd.tensor_scalar(
                                    yt_sb[:, o * 1024 + m0:o * 1024 + m0 + 512],
                                    et[:], bias_sb[:, o:o + 1], None, ALU.add)

            for c4 in range(4):
                nc.sync.dma_start(yt_out[:, c4 * 1024:(c4 + 1) * 1024],
                                  yt_sb[:, c4 * 1024:(c4 + 1) * 1024])

    nc.compile()
    return nc


def _prep_inputs(x, conditioning, w1, b1, w2, b2, bias):
    c = _consts()
    x = np.asarray(x, dtype=np.float32)
    conditioning = np.asarray(conditioning, dtype=np.float32)
    w1 = np.asarray(w1, dtype=np.float32)
    b1 = np.asarray(b1, dtype=np.float32)
    w2 = np.asarray(w2, dtype=np.float32)
    b2 = np.asarray(b2, dtype=np.float32)
    bias = np.asarray(bias, dtype=np.float32)

    w1t = w1.reshape(32, 32, 4, 64).transpose(1, 3, 2, 0).reshape(2048, 4, 32)
    w1t_sb = np.ascontiguousarray(
        w1t.reshape(16, 128, 4, 32).transpose(1, 0, 2, 3).reshape(128, 2048)
    ).astype(np.float16)
    w2t = np.ascontiguousarray(w2[:, :, 0].T).astype(np.float16)   # [32, 64]
    bias64 = np.tile(bias.reshape(1, 4), (64, 1)).astype(np.float32)

    shared = {
        "w1t_in": w1t_sb, "w2t_in": w2t,
        "b1_in": b1.reshape(32, 1).astype(np.float32),
        "b2a_in": b2[:32].reshape(32, 1).astype(np.float32),
        "b2w_in": b2[32:].reshape(32, 1).astype(np.float32),
        "e2z_in": c["e2z"], "e2a_in": c["e2a"],
        "m1a_in": c["m1a"], "m1b_in": c["m1b"],
        "wx_in": c["wx_h"], "ica_in": c["ica"], "icb_in": c["icb"],
        "invt_in": c["invt"], "bias_in": bias64, "ones_in": c["ones_row"],
    }
    L2 = 64 * H2
    in_maps = []
    for b in range(B):
        condpad = np.zeros((CI, L2), dtype=np.float32)
        condpad[:, 128:128 + T] = conditioning[b]
        d = condpad.reshape(CI, H2, 64).transpose(0, 2, 1).reshape(2048, H2)
        d = np.ascontiguousarray(d.reshape(16, 128, H2)).astype(np.float16)
        xp = np.zeros((2, L2), dtype=np.float32)
        xp[:, 0:T + 256] = np.pad(x[b], ((0, 0), (128, 128)), mode="reflect")
        xd = np.ascontiguousarray(
            xp.reshape(2, H2, 64).transpose(0, 2, 1).reshape(2, 64, H2)
            .transpose(1, 0, 2).reshape(64, 2 * H2)).astype(np.float16)
        m = dict(shared)
        m["d_in"] = d
        m["xd_in"] = xd
        in_maps.append(m)
    return in_maps


def _assemble(results):
    y = np.empty((B, O, T), dtype=np.float32)
    for b in range(B):
        yt = results[b]["yt_out"]                        # [64, 4096]
        y[b] = yt.reshape(64, 4, 1024).transpose(1, 2, 0).reshape(4, T)
    return y


def kernel(x, conditioning, w1, b1, w2, b2, bias):
    from concourse.bass_utils import run_bass_kernel_spmd
    if "nc" not in _prog_cache:
        _prog_cache["nc"] = _build_program()
    nc = _prog_cache["nc"]
    in_maps = _prep_inputs(x, conditioning, w1, b1, w2, b2, bias)
    res = run_bass_kernel_spmd(nc, in_maps, core_ids=list(range(B)))
    return _assemble(res.results)


# revision 28
# speedup vs baseline: 1.3515x; 1.3515x over previous
"""DynamicSincConv1d Trainium2 kernel (v2).

Data-parallel over batch: 8 batch elements -> 8 NeuronCores. All heavy
math runs on-device in fp16 matmuls (1 cyc/row on the PE) with fp32 PSUM
accumulation; elementwise work is fp16 on the DVE where possible.

Math notes (on top of the v1 symmetry tricks):
 - STFT/irFFT are matmuls against host-baked DFT matrices with windows
   folded in; the sinc bank is symmetric about k=128 so only d=|k-128|
   in [0,128) is materialized and every filter spectrum is real.
 - z = ta_d*wid + pi*eps is produced directly by the broadcast matmul:
   the one-hot selector is pre-scaled by ta_d and an appended ones-row
   carries the +pi*eps bias.
 - sinc(z) = P(z^2) (degree-3 minimax fit, |err| < 1e-6 over the full
   z range |z| <= pi/2) evaluated together with the amp multiply in a
   single custom DVE op: out = (u*(c1 + u*(c2 + u*c3)))*amp + amp.
 - The 4 sinc banks are pre-summed on the DVE so the filter DFT is a
   single 128x128 matmul stream per (oi, a/b) tile.
 - Overlap-add is folded into the iSTFT matmul accumulation: the four
   frame-shifted contributions accumulate in PSUM via column-shifted
   moving operands, then one env-normalize + bias pass finishes y.
 - F is padded 1025 -> 1026; the junk frame never reaches the output
   because the fused-OLA matmuls exclude it by construction.
"""

import math
import numpy as np

B, CI, I, O, S = 8, 32, 2, 4, 4
K, HOP, T = 256, 64, 65536
F = T // HOP + 1             # 1025 real frames
FE = F + 1                   # padded frame count (col 1025 is junk)
H2 = (T + K) // HOP + 2      # 1030 padded hop columns (frame f uses cols f..f+3)
PI = math.pi
PIEPS = PI * 1e-6
# degree-3 minimax fit of sinc(z) = 1 + u*(c1 + u*(c2 + u*c3)), u = z^2
SC1, SC2, SC3 = -0.16665935405036883, 0.008315297713095644, -0.00018570764930803712
M1S = 16.0                   # m1 scale (fp16 range); folded out via invt
ICS = 16.0                   # ic scale; folded out via invt
CH = [(0, 512), (512, 512)]  # main column chunks; tail cols [1024:1026]

_prog_cache = {}


def _register_sinc_amp():
    import concourse.dve_ops as dve_ops
    from concourse.dve_spec import Spec, Src0, Src1, C0, C1, C2, sq, lower
    from concourse.dve_uop import DveOpSpec

    for op in dve_ops.OPS:
        if op.name == "SINC_AMP_ANT":
            return op
    u = sq(Src0)
    body = (u * (C0 + u * (C1 + u * C2))) * Src1 + Src1

    def ref(in0, in1, s0, s1, imm2):
        uu = np.square(in0.astype(np.float32))
        return ((uu * (s0 + uu * (s1 + uu * imm2))) * in1 + in1).astype(np.float32)

    spec = Spec(body=body, reference=ref)
    row = max(dve_ops._SUB_OPCODE_FOR_NAME.values()) + 1
    assert row < 0x20
    shas = {}
    for ver in ("v3", "v4"):
        uops = lower(spec, ver=ver)
        shas[ver] = DveOpSpec(
            name="SINC_AMP_ANT", opcode=row, uops=uops, rd1_en=True
        ).sha(ver)
    op = dve_ops.DveOp("SINC_AMP_ANT", spec, subdim=False, uops_sha=shas)
    dve_ops.OPS.append(op)
    dve_ops.CUSTOM_DVE_SPECS["SINC_AMP_ANT"] = spec
    dve_ops._SUB_OPCODE_FOR_NAME["SINC_AMP_ANT"] = row
    return op


def _consts():
    n = np.arange(K, dtype=np.float64)
    ola = 0.5 * (1.0 - np.cos(2.0 * np.pi * n / K))
    fir = 0.42 - 0.5 * np.cos(2.0 * np.pi * n / K) + 0.08 * np.cos(4.0 * np.pi * n / K)

    d = np.arange(128, dtype=np.float64)
    ta = (PI * d / K)                                  # pi*d/256

    # M1[d, fb] = (-1)^fb * c_d * fir[128+d]/(S*K) * cos(2*pi*d*fb/K), x M1S
    fb = np.arange(K // 2 + 1, dtype=np.float64)       # 0..128
    cd = np.where(d == 0, 1.0, 2.0)
    m1 = (((-1.0) ** fb)[None, :] * cd[:, None] * fir[128 + d.astype(int)][:, None]
          / (S * K) * np.cos(2.0 * np.pi * np.outer(d, fb) / K)) * M1S
    m1a = np.ascontiguousarray(m1[:, 0:128]).astype(np.float16)
    m1b = np.concatenate([m1[:, 128:129], m1[:, 1:128]], axis=1).astype(np.float16)

    # STFT weights: wx[j][r, col]; k = 64 j + r; fbpack col layout
    kk = np.arange(K, dtype=np.float64)
    ang = 2.0 * np.pi * np.outer(kk, fb) / K           # [256, 129]
    wre = ola[:, None] * np.cos(ang)
    wim = -ola[:, None] * np.sin(ang)
    colsA = wre[:, 0:128]
    colsB = np.concatenate([wre[:, 128:129], wim[:, 1:128]], axis=1)
    wx_full = np.concatenate([colsA, colsB], axis=1)   # [256, 256]
    wx = wx_full.reshape(4, 64, 256)
    wx_h = np.ascontiguousarray(
        wx.transpose(1, 0, 2).reshape(64, 1024)).astype(np.float16)

    # iSTFT: IC[fbpack_row, n] with ola folded, x ICS
    cp = np.where(fb == 0, 1.0, 2.0)
    icre = (cp[:, None] / K) * np.cos(2.0 * np.pi * np.outer(fb, n) / K) * ola[None, :] * ICS
    icim = (-2.0 / K) * np.sin(2.0 * np.pi * np.outer(fb, n) / K) * ola[None, :] * ICS
    ica = np.ascontiguousarray(icre[0:128]).astype(np.float16)                    # [128, 256]
    icb = np.concatenate([icre[128:129], icim[1:128]], axis=0).astype(np.float16)  # [128, 256]

    # env inverse, arranged [r, p]; absorb 1/(M1S*ICS)
    ola2 = ola * ola
    env_q = np.zeros((1028, 64), dtype=np.float64)
    for j in range(4):
        env_q[j:F + j, :] += ola2[64 * j:64 * j + 64][None, :]
    invt = (1.0 / (env_q[2:2 + 1024, :] * (M1S * ICS))).T.astype(np.float32)
    invt = np.ascontiguousarray(invt)                  # [64, 1024]

    # z-broadcast selector [33, 4096]: rows q<32: (q==ois)*ta_d; row 32: pieps
    ta16 = ta.astype(np.float16)
    e2z = np.zeros((33, 32 * 128), dtype=np.float16)
    e2a = np.zeros((32, 32 * 128), dtype=np.float16)
    for q in range(32):
        e2z[q, q * 128:(q + 1) * 128] = ta16
        e2a[q, q * 128:(q + 1) * 128] = 1.0
    e2z[32, :] = np.float16(PIEPS)
    ones_row = np.ones((1, FE), dtype=np.float16)
    id128 = np.eye(128, dtype=np.float16)
    return dict(id128=id128, m1a=m1a, m1b=m1b, wx_h=wx_h, ica=ica, icb=icb, invt=invt,
                e2z=e2z, e2a=e2a, ones_row=ones_row)


def _build_program():
    import concourse.bacc as bacc
    import concourse.mybir as mybir
    import concourse.tile as tile

    sinc_op = _register_sinc_amp()

    f16 = mybir.dt.float16
    f32 = mybir.dt.float32
    AF = mybir.ActivationFunctionType
    ALU = mybir.AluOpType

    nc = bacc.Bacc("TRN2", target_bir_lowering=False, debug=False, num_devices=8)

    d_in = nc.dram_tensor("d_in", [128, 8 * 2112 + 96], f16, kind="ExternalInput")
    xd_in = nc.dram_tensor("xd_in", [64, 2 * H2], f16, kind="ExternalInput")
    w1t_in = nc.dram_tensor("w1t_in", [128, 2048], f16, kind="ExternalInput")
    w2t_in = nc.dram_tensor("w2t_in", [32, 64], f16, kind="ExternalInput")
    b1_in = nc.dram_tensor("b1_in", [32, 1], f32, kind="ExternalInput")
    b2a_in = nc.dram_tensor("b2a_in", [32, 1], f32, kind="ExternalInput")
    b2w_in = nc.dram_tensor("b2w_in", [32, 1], f32, kind="ExternalInput")
    e2z_in = nc.dram_tensor("e2z_in", [33, 4096], f16, kind="ExternalInput")
    e2a_in = nc.dram_tensor("e2a_in", [32, 4096], f16, kind="ExternalInput")
    m1a_in = nc.dram_tensor("m1a_in", [128, 128], f16, kind="ExternalInput")
    m1b_in = nc.dram_tensor("m1b_in", [128, 128], f16, kind="ExternalInput")
    wx_in = nc.dram_tensor("wx_in", [64, 1024], f16, kind="ExternalInput")
    ica_in = nc.dram_tensor("ica_in", [128, 256], f16, kind="ExternalInput")
    icb_in = nc.dram_tensor("icb_in", [128, 256], f16, kind="ExternalInput")
    invt_in = nc.dram_tensor("invt_in", [64, 1024], f32, kind="ExternalInput")
    bias_in = nc.dram_tensor("bias_in", [64, 4], f32, kind="ExternalInput")
    ones_in = nc.dram_tensor("ones_in", [1, FE], f16, kind="ExternalInput")
    id_in = nc.dram_tensor("id_in", [128, 128], f16, kind="ExternalInput")
    yt_out = nc.dram_tensor("yt_out", [64, 4096], f32, kind="ExternalOutput")

    with tile.TileContext(nc) as tc:
        with tc.tile_pool(name="cpool", bufs=1) as cpool:
            w1t_sb = cpool.tile([128, 2048], f16, tag="w1t")
            w2t_sb = cpool.tile([32, 64], f16, tag="w2t")
            b1_sb = cpool.tile([32, 1], f32, tag="b1")
            b2a_sb = cpool.tile([32, 1], f32, tag="b2a")
            b2w_sb = cpool.tile([32, 1], f32, tag="b2w")
            e2z_sb = cpool.tile([33, 4096], f16, tag="e2z")
            e2a_sb = cpool.tile([32, 4096], f16, tag="e2a")
            m1a_sb = cpool.tile([128, 128], f16, tag="m1a")
            m1b_sb = cpool.tile([128, 128], f16, tag="m1b")
            wx_sb = cpool.tile([64, 1024], f16, tag="wx")
            ica_sb = cpool.tile([128, 256], f16, tag="ica")
            icb_sb = cpool.tile([128, 256], f16, tag="icb")
            invt_sb = cpool.tile([64, 1024], f32, tag="invt")
            bias_sb = cpool.tile([64, 4], f32, tag="bias")
            xd_sb = cpool.tile([64, 2 * H2], f16, tag="xd")
            h_sb = cpool.tile([32, FE], f16, tag="h")
            wa_sb = cpool.tile([33, 2 * FE], f16, tag="wa")
            xa_sb = cpool.tile([128, 2 * FE], f16, tag="xa")
            xb_sb = cpool.tile([128, 2 * FE], f16, tag="xb")
            yt_sb = cpool.tile([64, 4096], f32, tag="yt")
            id_sb = cpool.tile([128, 128], f16, tag="id128")

            # load order matters: stage-1 operands first, spread over the
            # two HWDGE queues (SP + Activation)
            nc.sync.dma_start(w1t_sb[:], w1t_in[:])
            nc.scalar.dma_start(b1_sb[:], b1_in[:])
            nc.scalar.dma_start(id_sb[:], id_in[:])

            # ---- stage 1: conditioning conv -> h [32, FE] fp16 ----
            with tc.tile_pool(name="dpool", bufs=1) as dpool, \
                 tc.tile_pool(name="hts", bufs=2) as htsp, \
                 tc.tile_pool(name="ps1", bufs=2, space="PSUM") as ps1, \
                 tc.tile_pool(name="ps2", bufs=2, space="PSUM") as ps2, \
                 tc.tile_pool(name="psT", bufs=2, space="PSUM") as psT:
                # band-major d: band k holds cols [k*128, k*128+132) of all
                # 16 c-chunks contiguously; ftile k's matmuls start as soon as
                # band k lands.
                dbig = dpool.tile([128, 8 * 2112 + 96], f16, tag="dbig")
                BW = [2112] * 8 + [96]
                boff = [0]
                for w in BW:
                    boff.append(boff[-1] + w)
                for k in range(9):
                    nc.sync.dma_start(dbig[:, boff[k]:boff[k + 1]],
                                      d_in[:, boff[k]:boff[k + 1]])
                for t_sb, t_in in (
                        (w2t_sb, w2t_in), (b2w_sb, b2w_in), (b2a_sb, b2a_in),
                        (e2z_sb, e2z_in), (e2a_sb, e2a_in), (xd_sb, xd_in),
                        (wx_sb, wx_in), (m1a_sb, m1a_in), (m1b_sb, m1b_in),
                        (ica_sb, ica_in), (icb_sb, icb_in), (invt_sb, invt_in),
                        (bias_sb, bias_in)):
                    nc.sync.dma_start(t_sb[:], t_in[:])
                nc.sync.dma_start(wa_sb[32:33, 0:FE], ones_in[:])
                FT1 = [(k, 128) for k in range(0, 1024, 128)] + [(1024, 2)]
                for ft, (f0, nf) in enumerate(FT1):
                    cw = 132 if ft < 8 else 6
                    ps = ps1.tile([128, 32], f32, tag="ps1")
                    idx = 0
                    for c in range(16):
                        for j in range(4):
                            nc.tensor.matmul(
                                ps[0:nf, :],
                                dbig[:, boff[ft] + c * cw + j:
                                     boff[ft] + c * cw + j + nf],
                                w1t_sb[:, (c * 4 + j) * 32:(c * 4 + j + 1) * 32],
                                start=(idx == 0), stop=(idx == 63))
                            idx += 1
                    ht16 = htsp.tile([128, 32], f16, tag="ht16")
                    nc.scalar.activation(ht16[0:nf, :], ps[0:nf, :], AF.Copy)
                    pt = psT.tile([32, 128], f16, tag="psT")
                    nc.tensor.matmul(pt[:, 0:nf], ht16[0:nf, :],
                                     id_sb[0:nf, 0:nf],
                                     start=True, stop=True, is_transpose=True)
                    nc.scalar.activation(h_sb[:, f0:f0 + nf], pt[:, 0:nf],
                                         AF.Identity, bias=b1_sb[:, 0:1])
                    # leaky-relu + stage 2 chunkwise as soon as h cols land
                    if ft in (3, 7, 8):
                        c0 = {3: 0, 7: 512, 8: 1024}[ft]
                        nc_ = {3: 512, 7: 512, 8: 2}[ft]
                        lt = dpool.tile([32, 512], f16, tag="lt")
                        nc.vector.tensor_scalar(lt[:, 0:nc_], h_sb[:, c0:c0 + nc_],
                                                0.01, None, ALU.mult)
                        nc.vector.tensor_max(h_sb[:, c0:c0 + nc_],
                                             h_sb[:, c0:c0 + nc_], lt[:, 0:nc_])
                        pw = ps2.tile([32, nc_ if nc_ > 2 else 2], f32, tag="ps2w")
                        nc.tensor.matmul(pw[:], w2t_sb[:, 32:64],
                                         h_sb[:, c0:c0 + nc_],
                                         start=True, stop=True)
                        nc.scalar.activation(wa_sb[0:32, c0:c0 + nc_], pw[:],
                                             AF.Tanh, bias=b2w_sb[:, 0:1])
                        pa = ps2.tile([32, nc_ if nc_ > 2 else 2], f32, tag="ps2a")
                        nc.tensor.matmul(pa[:], w2t_sb[:, 0:32],
                                         h_sb[:, c0:c0 + nc_],
                                         start=True, stop=True)
                        nc.scalar.activation(wa_sb[0:32, FE + c0:FE + c0 + nc_],
                                             pa[:], AF.Tanh, bias=b2a_sb[:, 0:1])

            # ---- global tail pre-pass: z/amp/X tail columns for all ois ----
            ftg = cpool.tile([128, 64], f16, tag="ftg")
            with tc.tile_pool(name="gtp", bufs=1, space="PSUM") as gtp:
                gt = gtp.tile([128, 136], f32, tag="gt")
                for ois in range(32):
                    esl = slice(ois * 128, (ois + 1) * 128)
                    nc.tensor.matmul(gt[:, ois * 2:ois * 2 + 2],
                                     e2z_sb[:, esl], wa_sb[0:33, 1024:1026],
                                     start=True, stop=True)
                    nc.tensor.matmul(gt[:, 64 + ois * 2:64 + ois * 2 + 2],
                                     e2a_sb[:, esl],
                                     wa_sb[0:32, FE + 1024:FE + 1026],
                                     start=True, stop=True)
                for i in range(2):
                    for mt in range(2):
                        c0 = 128 + (i * 2 + mt) * 2
                        for j in range(4):
                            nc.tensor.matmul(
                                gt[:, c0:c0 + 2],
                                wx_sb[:, j * 256 + mt * 128: j * 256 + mt * 128 + 128],
                                xd_sb[:, i * H2 + 1024 + j:i * H2 + 1026 + j],
                                start=(j == 0), stop=(j == 3))
                a16g = cpool.tile([128, 64], f16, tag="a16g")
                nc.scalar.activation(a16g[:], gt[:, 64:128], AF.Copy)
                nc.vector._custom_dve(
                    sinc_op, out=ftg[:], in0=gt[:, 0:64], in1=a16g[:],
                    s0=SC1, s1=SC2, imm2=SC3)
                xa_r = xa_sb[:].rearrange("p (i f) -> p i f", i=2)
                xb_r = xb_sb[:].rearrange("p (i f) -> p i f", i=2)
                nc.scalar.activation(
                    xa_r[:, :, 1024:1026],
                    gt[:, 128:136].rearrange("p (i mt t) -> p i (mt t)",
                                             i=2, mt=2)[:, :, 0:2], AF.Copy)
                nc.scalar.activation(
                    xb_r[:, :, 1024:1026],
                    gt[:, 128:136].rearrange("p (i mt t) -> p i (mt t)",
                                             i=2, mt=2)[:, :, 2:4], AF.Copy)

            # ---- per half: sinc synth + DFT + cmul + fused iSTFT/OLA ----
            # stage-4 units (i, mt) are interleaved into the half-0 oi loop so
            # the PE fills DVE-bound sinc-phase gaps.
            s4units = [(i, mt, xdst) for i in range(2)
                       for (mt, xdst) in ((0, xa_sb), (1, xb_sb))]
            # (consumed one per DFT emission in half 0)
            for half in range(2):
                with tc.tile_pool(name="fp", bufs=1) as fppool:
                    # fpab layout: [m(2)][oil(4)][FE] filter spectra, fp16
                    fpab = fppool.tile([128, 8 * FE], f16, tag="fpab")
                    with tc.tile_pool(name="ftp", bufs=2) as ftpool, \
                         tc.tile_pool(name="fsp", bufs=2) as fspool, \
                         tc.tile_pool(name="a16p", bufs=2) as a16p, \
                         tc.tile_pool(name="zps", bufs=2, space="PSUM") as zps, \
                         tc.tile_pool(name="aps", bufs=2, space="PSUM") as aps:
                        def emit_dft(oil, fsum, dtl):
                            for (mi, m1sb) in ((0, m1a_sb), (1, m1b_sb)):
                                dp = zps.tile([128, 1024], f32, tag="zp")
                                for (f0, nf) in CH:
                                    nc.tensor.matmul(dp[:, f0:f0 + nf], m1sb[:],
                                                     fsum[:, f0:f0 + nf],
                                                     start=True, stop=True)
                                nc.tensor.matmul(dtl[:, mi * 2:mi * 2 + 2], m1sb[:],
                                                 fsum[:, 1024:1026],
                                                 start=True, stop=True)
                                nc.scalar.activation(
                                    fpab[:, (mi * 4 + oil) * FE:(mi * 4 + oil) * FE + 1024],
                                    dp[:], AF.Copy)
                            fp_r = fpab[:].rearrange("p (m f) -> p m f", m=2)
                            nc.scalar.activation(
                                fp_r[:, :, oil * FE + 1024:oil * FE + 1026],
                                dtl[:, 0:4].rearrange("p (m t) -> p m t", m=2),
                                AF.Copy)
                            # one stage-4 STFT unit rides along (half 0)
                            if half == 0 and s4units:
                                i, mt, xdst = s4units.pop(0)
                                x4 = zps.tile([128, 1024], f32, tag="zp")
                                for (f0, nf) in CH:
                                    for j in range(4):
                                        nc.tensor.matmul(
                                            x4[:, f0:f0 + nf],
                                            wx_sb[:, j * 256 + mt * 128: j * 256 + mt * 128 + 128],
                                            xd_sb[:, i * H2 + f0 + j:i * H2 + f0 + j + nf],
                                            start=(j == 0), stop=(j == 3))
                                nc.scalar.activation(
                                    xdst[:, i * FE:i * FE + 1024], x4[:], AF.Copy)

                        pending = None
                        for oil in range(4):
                            oi = half * 4 + oil
                            ftoi = ftpool.tile([128, 4 * FE], f16, tag="ftoi")
                            # tail columns from the global pre-pass (Pool)
                            ft_r = ftoi[:].rearrange("p (s f) -> p s f", s=4)
                            nc.gpsimd.tensor_scalar(
                                ft_r[:, :, 1024:1026], ftg[:, oi * 8:oi * 8 + 8],
                                1.0, None, ALU.mult)
                            for s in range(4):
                                ois = oi * 4 + s
                                esl = slice(ois * 128, (ois + 1) * 128)
                                ap = aps.tile([128, 1024], f32, tag="ap")
                                a16 = a16p.tile([128, 1024], f16, tag="a16")
                                zp = zps.tile([128, 1024], f32, tag="zp")
                                for (f0, nf) in CH:
                                    nc.tensor.matmul(
                                        zp[:, f0:f0 + nf],
                                        e2z_sb[:, esl], wa_sb[0:33, f0:f0 + nf],
                                        start=True, stop=True)
                                for (f0, nf) in CH:
                                    nc.tensor.matmul(
                                        ap[:, f0:f0 + nf],
                                        e2a_sb[:, esl],
                                        wa_sb[0:32, FE + f0:FE + f0 + nf],
                                        start=True, stop=True)
                                nc.scalar.activation(a16[:], ap[:], AF.Copy)
                                nc.vector._custom_dve(
                                    sinc_op,
                                    out=ftoi[:, s * FE:s * FE + 1024],
                                    in0=zp[:], in1=a16[:],
                                    s0=SC1, s1=SC2, imm2=SC3)

                            # ---- s-presum: ftsum = sum_s ft_s (all DVE) ----
                            fs1 = fspool.tile([128, FE], f16, tag="fs1")
                            fsum = fspool.tile([128, FE], f16, tag="fsum")
                            nc.gpsimd.tensor_add(fs1[:], ftoi[:, 0:FE],
                                                 ftoi[:, FE:2 * FE])
                            nc.vector.tensor_add(fsum[:], ftoi[:, 2 * FE:3 * FE],
                                                 ftoi[:, 3 * FE:4 * FE])
                            nc.vector.tensor_add(fsum[:], fsum[:], fs1[:])
                            dtl = aps.tile([128, 1024], f32, tag="ap")

                            # software pipeline: previous oi's DFT goes behind
                            # this oi's sinc matmuls so the PE never blocks on
                            # the presum chain.
                            if pending is not None:
                                emit_dft(*pending)
                            pending = (oil, fsum, dtl)
                        emit_dft(*pending)

                    # ---- stage 5 + fused iSTFT/OLA + env/bias ----
                    with tc.tile_pool(name="yp5", bufs=2) as ypool, \
                         tc.tile_pool(name="tt5", bufs=2) as ttpool, \
                         tc.tile_pool(name="ops", bufs=2, space="PSUM") as ops, \
                         tc.tile_pool(name="etp", bufs=2) as etpool:
                        for ol in range(2):
                            o = half * 2 + ol
                            ya = ypool.tile([128, FE], f16, tag="ya")
                            yb = ypool.tile([128, FE], f16, tag="yb")
                            ta_t = ttpool.tile([128, 2 * FE], f16, tag="tat")
                            nc.vector.tensor_mul(
                                ta_t[:], xa_sb[:],
                                fpab[:, (ol * 2) * FE:(ol * 2 + 2) * FE])
                            nc.vector.tensor_add(ya[:], ta_t[:, 0:FE],
                                                 ta_t[:, FE:2 * FE])
                            tb_t = ttpool.tile([128, 2 * FE], f16, tag="tbt")
                            nc.vector.tensor_mul(
                                tb_t[:], xb_sb[:],
                                fpab[:, (4 + ol * 2) * FE:(4 + ol * 2 + 2) * FE])
                            nc.vector.tensor_add(yb[:], tb_t[:, 0:FE],
                                                 tb_t[:, FE:2 * FE])

                            for c, m0 in ((0, 0), (1, 512)):
                                yp = ops.tile([64, 512], f32, tag="yp")
                                # pick a j whose column window is full-width to
                                # carry start=True; trimmed edges accumulate.
                                jorder = (0, 1, 2, 3) if c == 0 else (1, 0, 2, 3)
                                first = True
                                n_mm = 8
                                k = 0
                                for j in jorder:
                                    lo = m0 + 2 - j          # ya col of out col 0
                                    o0, o1 = 0, 512          # out col range
                                    if lo < 0:
                                        o0 = -lo
                                    if lo + 512 > F:
                                        o1 = F - lo
                                    for ic_sb, ysrc in ((ica_sb, ya), (icb_sb, yb)):
                                        k += 1
                                        nc.tensor.matmul(
                                            yp[:, o0:o1],
                                            ic_sb[:, j * 64:(j + 1) * 64],
                                            ysrc[:, lo + o0:lo + o1],
                                            start=first, stop=(k == n_mm),
                                            skip_group_check=True)
                                        first = False
                                et = etpool.tile([64, 512], f32, tag="et")
                                nc.vector.tensor_mul(et[:], yp[:],
                                                     invt_sb[:, m0:m0 + 512])
                                nc.gpsimd.tensor_scalar(
                                    yt_sb[:, o * 1024 + m0:o * 1024 + m0 + 512],
                                    et[:], bias_sb[:, o:o + 1], None, ALU.add)
                                nc.sync.dma_start(
                                    yt_out[:, o * 1024 + m0:o * 1024 + m0 + 512],
                                    yt_sb[:, o * 1024 + m0:o * 1024 + m0 + 512])


    nc.compile()
    return nc


def _prep_inputs(x, conditioning, w1, b1, w2, b2, bias):
    c = _consts()
    x = np.asarray(x, dtype=np.float32)
    conditioning = np.asarray(conditioning, dtype=np.float32)
    w1 = np.asarray(w1, dtype=np.float32)
    b1 = np.asarray(b1, dtype=np.float32)
    w2 = np.asarray(w2, dtype=np.float32)
    b2 = np.asarray(b2, dtype=np.float32)
    bias = np.asarray(bias, dtype=np.float32)

    w1t = w1.reshape(32, 32, 4, 64).transpose(1, 3, 2, 0).reshape(2048, 4, 32)
    w1t_sb = np.ascontiguousarray(
        w1t.reshape(16, 128, 4, 32).transpose(1, 0, 2, 3).reshape(128, 2048)
    ).astype(np.float16)
    w2t = np.ascontiguousarray(w2[:, :, 0].T).astype(np.float16)   # [32, 64]
    bias64 = np.tile(bias.reshape(1, 4), (64, 1)).astype(np.float32)

    shared = {
        "w1t_in": w1t_sb, "w2t_in": w2t,
        "b1_in": b1.reshape(32, 1).astype(np.float32),
        "b2a_in": b2[:32].reshape(32, 1).astype(np.float32),
        "b2w_in": b2[32:].reshape(32, 1).astype(np.float32),
        "e2z_in": c["e2z"], "e2a_in": c["e2a"],
        "m1a_in": c["m1a"], "m1b_in": c["m1b"],
        "wx_in": c["wx_h"], "ica_in": c["ica"], "icb_in": c["icb"],
        "invt_in": c["invt"], "bias_in": bias64, "ones_in": c["ones_row"],
        "id_in": c["id128"],
    }
    L2 = 64 * H2
    in_maps = []
    for b in range(B):
        condpad = np.zeros((CI, L2), dtype=np.float32)
        condpad[:, 128:128 + T] = conditioning[b]
        d = condpad.reshape(CI, H2, 64).transpose(0, 2, 1).reshape(2048, H2)
        d = d.reshape(16, 128, H2)                     # [c, p, col]
        parts = []
        for k in range(8):
            parts.append(d[:, :, k * 128:k * 128 + 132])
        parts.append(d[:, :, 1024:1030])
        d = np.concatenate([p.transpose(1, 0, 2).reshape(128, -1)
                            for p in parts], axis=1)
        d = np.ascontiguousarray(d).astype(np.float16)
        xp = np.zeros((2, L2), dtype=np.float32)
        xp[:, 0:T + 256] = np.pad(x[b], ((0, 0), (128, 128)), mode="reflect")
        xd = np.ascontiguousarray(
            xp.reshape(2, H2, 64).transpose(0, 2, 1).reshape(2, 64, H2)
            .transpose(1, 0, 2).reshape(64, 2 * H2)).astype(np.float16)
        m = dict(shared)
        m["d_in"] = d
        m["xd_in"] = xd
        in_maps.append(m)
    return in_maps


def _assemble(results):
    y = np.empty((B, O, T), dtype=np.float32)
    for b in range(B):
        yt = results[b]["yt_out"]                        # [64, 4096]
        y[b] = yt.reshape(64, 4, 1024).transpose(1, 2, 0).reshape(4, T)
    return y


def kernel(x, conditioning, w1, b1, w2, b2, bias):
    from concourse.bass_utils import run_bass_kernel_spmd
    if "nc" not in _prog_cache:
        _prog_cache["nc"] = _build_program()
    nc = _prog_cache["nc"]
    in_maps = _prep_inputs(x, conditioning, w1, b1, w2, b2, bias)
    res = run_bass_kernel_spmd(nc, in_maps, core_ids=list(range(B)))
    return _assemble(res.results)


# revision 55
# speedup vs baseline: 1.4872x; 1.1004x over previous
"""DynamicSincConv1d Trainium2 kernel (v2).

Data-parallel over batch: 8 batch elements -> 8 NeuronCores. All heavy
math runs on-device in fp16 matmuls (1 cyc/row on the PE) with fp32 PSUM
accumulation; elementwise work is fp16 on the DVE where possible.

Math notes (on top of the v1 symmetry tricks):
 - STFT/irFFT are matmuls against host-baked DFT matrices with windows
   folded in; the sinc bank is symmetric about k=128 so only d=|k-128|
   in [0,128) is materialized and every filter spectrum is real.
 - z = ta_d*wid + pi*eps is produced directly by the broadcast matmul:
   the one-hot selector is pre-scaled by ta_d and an appended ones-row
   carries the +pi*eps bias.
 - sinc(z) = P(z^2) (degree-3 minimax fit, |err| < 1e-6 over the full
   z range |z| <= pi/2) evaluated together with the amp multiply in a
   single custom DVE op: out = (u*(c1 + u*(c2 + u*c3)))*amp + amp.
 - The 4 sinc banks are pre-summed on the DVE so the filter DFT is a
   single 128x128 matmul stream per (oi, a/b) tile.
 - Overlap-add is folded into the iSTFT matmul accumulation: the four
   frame-shifted contributions accumulate in PSUM via column-shifted
   moving operands, then one env-normalize + bias pass finishes y.
 - F is padded 1025 -> 1026; the junk frame never reaches the output
   because the fused-OLA matmuls exclude it by construction.
"""

import math
import numpy as np

B, CI, I, O, S = 8, 32, 2, 4, 4
K, HOP, T = 256, 64, 65536
F = T // HOP + 1             # 1025 real frames
FE = F + 1                   # padded frame count (col 1025 is junk)
H2 = (T + K) // HOP + 2      # 1030 padded hop columns (frame f uses cols f..f+3)
PI = math.pi
PIEPS = PI * 1e-6
# degree-3 minimax fit of sinc(z) = 1 + u*(c1 + u*(c2 + u*c3)), u = z^2
SC1, SC2, SC3 = -0.16665935405036883, 0.008315297713095644, -0.00018570764930803712
M1S = 16.0                   # m1 scale (fp16 range); folded out via invt
ICS = 16.0                   # ic scale; folded out via invt
CH = [(0, 512), (512, 512)]  # main column chunks; tail cols [1024:1026]

_prog_cache = {}


def _register_sinc_amp():
    import concourse.dve_ops as dve_ops
    from concourse.dve_spec import Spec, Src0, Src1, C0, C1, C2, sq, lower
    from concourse.dve_uop import DveOpSpec

    for op in dve_ops.OPS:
        if op.name == "SINC_AMP_ANT":
            return op
    u = sq(Src0)
    body = (u * (C0 + u * (C1 + u * C2))) * Src1 + Src1

    def ref(in0, in1, s0, s1, imm2):
        uu = np.square(in0.astype(np.float32))
        return ((uu * (s0 + uu * (s1 + uu * imm2))) * in1 + in1).astype(np.float32)

    spec = Spec(body=body, reference=ref)
    row = max(dve_ops._SUB_OPCODE_FOR_NAME.values()) + 1
    assert row < 0x20
    shas = {}
    for ver in ("v3", "v4"):
        uops = lower(spec, ver=ver)
        shas[ver] = DveOpSpec(
            name="SINC_AMP_ANT", opcode=row, uops=uops, rd1_en=True
        ).sha(ver)
    op = dve_ops.DveOp("SINC_AMP_ANT", spec, subdim=False, uops_sha=shas)
    dve_ops.OPS.append(op)
    dve_ops.CUSTOM_DVE_SPECS["SINC_AMP_ANT"] = spec
    dve_ops._SUB_OPCODE_FOR_NAME["SINC_AMP_ANT"] = row
    return op


def _consts():
    n = np.arange(K, dtype=np.float64)
    ola = 0.5 * (1.0 - np.cos(2.0 * np.pi * n / K))
    fir = 0.42 - 0.5 * np.cos(2.0 * np.pi * n / K) + 0.08 * np.cos(4.0 * np.pi * n / K)

    d = np.arange(128, dtype=np.float64)
    ta = (PI * d / K)                                  # pi*d/256

    # M1[d, fb] = (-1)^fb * c_d * fir[128+d]/(S*K) * cos(2*pi*d*fb/K), x M1S
    fb = np.arange(K // 2 + 1, dtype=np.float64)       # 0..128
    cd = np.where(d == 0, 1.0, 2.0)
    m1 = (((-1.0) ** fb)[None, :] * cd[:, None] * fir[128 + d.astype(int)][:, None]
          / (S * K) * np.cos(2.0 * np.pi * np.outer(d, fb) / K)) * M1S
    m1a = np.ascontiguousarray(m1[:, 0:128]).astype(np.float16)
    m1b = np.concatenate([m1[:, 128:129], m1[:, 1:128]], axis=1).astype(np.float16)

    # STFT weights: wx[j][r, col]; k = 64 j + r; fbpack col layout
    kk = np.arange(K, dtype=np.float64)
    ang = 2.0 * np.pi * np.outer(kk, fb) / K           # [256, 129]
    wre = ola[:, None] * np.cos(ang)
    wim = -ola[:, None] * np.sin(ang)
    colsA = wre[:, 0:128]
    colsB = np.concatenate([wre[:, 128:129], wim[:, 1:128]], axis=1)
    wx_full = np.concatenate([colsA, colsB], axis=1)   # [256, 256]
    wx = wx_full.reshape(4, 64, 256)
    wx_h = np.ascontiguousarray(
        wx.transpose(1, 0, 2).reshape(64, 1024)).astype(np.float16)

    # iSTFT: IC[fbpack_row, n] with ola folded, x ICS
    cp = np.where(fb == 0, 1.0, 2.0)
    icre = (cp[:, None] / K) * np.cos(2.0 * np.pi * np.outer(fb, n) / K) * ola[None, :] * ICS
    icim = (-2.0 / K) * np.sin(2.0 * np.pi * np.outer(fb, n) / K) * ola[None, :] * ICS
    ica = np.ascontiguousarray(icre[0:128]).astype(np.float16)                    # [128, 256]
    icb = np.concatenate([icre[128:129], icim[1:128]], axis=0).astype(np.float16)  # [128, 256]

    # env inverse, arranged [r, p]; absorb 1/(M1S*ICS)
    ola2 = ola * ola
    env_q = np.zeros((1028, 64), dtype=np.float64)
    for j in range(4):
        env_q[j:F + j, :] += ola2[64 * j:64 * j + 64][None, :]
    invt = (1.0 / (env_q[2:2 + 1024, :] * (M1S * ICS))).T.astype(np.float32)
    invt = np.ascontiguousarray(invt)                  # [64, 1024]

    # z-broadcast selector [33, 4096]: rows q<32: (q==ois)*ta_d; row 32: pieps
    ta16 = ta.astype(np.float16)
    e2z = np.zeros((33, 32 * 128), dtype=np.float16)
    e2a = np.zeros((32, 32 * 128), dtype=np.float16)
    for q in range(32):
        e2z[q, q * 128:(q + 1) * 128] = ta16
        e2a[q, q * 128:(q + 1) * 128] = 1.0
    e2z[32, :] = np.float16(PIEPS)
    ones_row = np.ones((1, FE), dtype=np.float16)
    id128 = np.eye(128, dtype=np.float16)
    return dict(id128=id128, m1a=m1a, m1b=m1b, wx_h=wx_h, ica=ica, icb=icb, invt=invt,
                e2z=e2z, e2a=e2a, ones_row=ones_row)


def _build_program():
    import concourse.bacc as bacc
    import concourse.mybir as mybir
    import concourse.tile as tile

    sinc_op = _register_sinc_amp()

    f16 = mybir.dt.float16
    f32 = mybir.dt.float32
    AF = mybir.ActivationFunctionType
    ALU = mybir.AluOpType

    nc = bacc.Bacc("TRN2", target_bir_lowering=False, debug=False, num_devices=8)

    d_in = nc.dram_tensor("d_in", [128, 8 * 2112 + 96], f16, kind="ExternalInput")
    xd_in = nc.dram_tensor("xd_in", [64, 2 * H2], f16, kind="ExternalInput")
    w1t_in = nc.dram_tensor("w1t_in", [128, 2048], f16, kind="ExternalInput")
    w2t_in = nc.dram_tensor("w2t_in", [32, 64], f16, kind="ExternalInput")
    b1_in = nc.dram_tensor("b1_in", [32, 1], f32, kind="ExternalInput")
    b2a_in = nc.dram_tensor("b2a_in", [32, 1], f32, kind="ExternalInput")
    b2w_in = nc.dram_tensor("b2w_in", [32, 1], f32, kind="ExternalInput")
    e2z_in = nc.dram_tensor("e2z_in", [33, 4096], f16, kind="ExternalInput")
    e2a_in = nc.dram_tensor("e2a_in", [32, 4096], f16, kind="ExternalInput")
    m1a_in = nc.dram_tensor("m1a_in", [128, 128], f16, kind="ExternalInput")
    m1b_in = nc.dram_tensor("m1b_in", [128, 128], f16, kind="ExternalInput")
    wx_in = nc.dram_tensor("wx_in", [64, 1024], f16, kind="ExternalInput")
    ica_in = nc.dram_tensor("ica_in", [128, 256], f16, kind="ExternalInput")
    icb_in = nc.dram_tensor("icb_in", [128, 256], f16, kind="ExternalInput")
    invt_in = nc.dram_tensor("invt_in", [64, 1024], f32, kind="ExternalInput")
    bias_in = nc.dram_tensor("bias_in", [64, 4], f32, kind="ExternalInput")
    ones_in = nc.dram_tensor("ones_in", [1, FE], f16, kind="ExternalInput")
    id_in = nc.dram_tensor("id_in", [128, 128], f16, kind="ExternalInput")
    yt_out = nc.dram_tensor("yt_out", [64, 4096], f32, kind="ExternalOutput")

    with tile.TileContext(nc) as tc:
        with tc.tile_pool(name="cpool", bufs=1) as cpool:
            w1t_sb = cpool.tile([128, 2048], f16, tag="w1t")
            w2t_sb = cpool.tile([32, 64], f16, tag="w2t")
            b1_sb = cpool.tile([32, 1], f32, tag="b1")
            b2a_sb = cpool.tile([32, 1], f32, tag="b2a")
            b2w_sb = cpool.tile([32, 1], f32, tag="b2w")
            e2z_sb = cpool.tile([33, 4096], f16, tag="e2z")
            e2a_sb = cpool.tile([32, 4096], f16, tag="e2a")
            m1a_sb = cpool.tile([128, 128], f16, tag="m1a")
            m1b_sb = cpool.tile([128, 128], f16, tag="m1b")
            wx_sb = cpool.tile([64, 1024], f16, tag="wx")
            ica_sb = cpool.tile([128, 256], f16, tag="ica")
            icb_sb = cpool.tile([128, 256], f16, tag="icb")
            invt_sb = cpool.tile([64, 1024], f32, tag="invt")
            bias_sb = cpool.tile([64, 4], f32, tag="bias")
            xd_sb = cpool.tile([64, 2 * H2], f16, tag="xd")
            h_sb = cpool.tile([32, FE], f16, tag="h")
            wa_sb = cpool.tile([33, 2 * FE], f16, tag="wa")
            xa_sb = cpool.tile([128, 2 * FE], f16, tag="xa")
            xb_sb = cpool.tile([128, 2 * FE], f16, tag="xb")
            yt_sb = cpool.tile([64, 4096], f32, tag="yt")
            id_sb = cpool.tile([128, 128], f16, tag="id128")

            # load order matters: stage-1 operands first, spread over the
            # two HWDGE queues (SP + Activation)
            nc.sync.dma_start(w1t_sb[:], w1t_in[:])
            nc.scalar.dma_start(b1_sb[:], b1_in[:])
            nc.scalar.dma_start(id_sb[:], id_in[:])
            nc.scalar.dma_start(wa_sb[32:33, 0:FE], ones_in[:])

            # ---- stage 1: conditioning conv -> h [32, FE] fp16 ----
            with tc.tile_pool(name="dpool", bufs=1) as dpool, \
                 tc.tile_pool(name="hts", bufs=2) as htsp, \
                 tc.tile_pool(name="ps1", bufs=2, space="PSUM") as ps1, \
                 tc.tile_pool(name="ps2", bufs=2, space="PSUM") as ps2, \
                 tc.tile_pool(name="psT", bufs=2, space="PSUM") as psT:
                # band-major d: band k holds cols [k*128, k*128+132) of all
                # 16 c-chunks contiguously; ftile k's matmuls start as soon as
                # band k lands.
                dbig = dpool.tile([128, 8 * 2112 + 96], f16, tag="dbig")
                BW = [2112] * 8 + [96]
                boff = [0]
                for w in BW:
                    boff.append(boff[-1] + w)
                for k in range(9):
                    nc.sync.dma_start(dbig[:, boff[k]:boff[k + 1]],
                                      d_in[:, boff[k]:boff[k + 1]])
                for t_sb, t_in in (
                        (w2t_sb, w2t_in), (b2w_sb, b2w_in), (b2a_sb, b2a_in),
                        (xd_sb, xd_in), (e2z_sb, e2z_in), (e2a_sb, e2a_in),
                        (wx_sb, wx_in),
                        (m1a_sb, m1a_in), (m1b_sb, m1b_in),
                        (ica_sb, ica_in), (icb_sb, icb_in), (invt_sb, invt_in),
                        (bias_sb, bias_in)):
                    nc.sync.dma_start(t_sb[:], t_in[:])
                FT1 = [(k, 128) for k in range(0, 1024, 128)] + [(1024, 2)]
                for ft, (f0, nf) in enumerate(FT1):
                    cw = 132 if ft < 8 else 6
                    ps = ps1.tile([128, 32], f32, tag="ps1")
                    idx = 0
                    for c in range(16):
                        for j in range(4):
                            nc.tensor.matmul(
                                ps[0:nf, :],
                                dbig[:, boff[ft] + c * cw + j:
                                     boff[ft] + c * cw + j + nf],
                                w1t_sb[:, (c * 4 + j) * 32:(c * 4 + j + 1) * 32],
                                start=(idx == 0), stop=(idx == 63))
                            idx += 1
                    ht16 = htsp.tile([128, 32], f16, tag="ht16")
                    nc.scalar.activation(ht16[0:nf, :], ps[0:nf, :], AF.Copy)
                    pt = psT.tile([32, 128], f16, tag="psT")
                    nc.tensor.matmul(pt[:, 0:nf], ht16[0:nf, :],
                                     id_sb[0:nf, 0:nf],
                                     start=True, stop=True, is_transpose=True)
                    nc.scalar.activation(h_sb[:, f0:f0 + nf], pt[:, 0:nf],
                                         AF.Identity, bias=b1_sb[:, 0:1])
                    # leaky-relu + stage 2 chunkwise as soon as h cols land
                    if ft in (3, 7, 8):
                        c0 = {3: 0, 7: 512, 8: 1024}[ft]
                        nc_ = {3: 512, 7: 512, 8: 2}[ft]
                        lt = dpool.tile([32, 512], f16, tag="lt")
                        nc.vector.tensor_scalar(lt[:, 0:nc_], h_sb[:, c0:c0 + nc_],
                                                0.01, None, ALU.mult)
                        nc.vector.tensor_max(h_sb[:, c0:c0 + nc_],
                                             h_sb[:, c0:c0 + nc_], lt[:, 0:nc_])
                        pw = ps2.tile([32, nc_ if nc_ > 2 else 2], f32, tag="ps2w")
                        nc.tensor.matmul(pw[:], w2t_sb[:, 32:64],
                                         h_sb[:, c0:c0 + nc_],
                                         start=True, stop=True)
                        nc.scalar.activation(wa_sb[0:32, c0:c0 + nc_], pw[:],
                                             AF.Tanh, bias=b2w_sb[:, 0:1])
                        pa = ps2.tile([32, nc_ if nc_ > 2 else 2], f32, tag="ps2a")
                        nc.tensor.matmul(pa[:], w2t_sb[:, 0:32],
                                         h_sb[:, c0:c0 + nc_],
                                         start=True, stop=True)
                        nc.scalar.activation(wa_sb[0:32, FE + c0:FE + c0 + nc_],
                                             pa[:], AF.Tanh, bias=b2a_sb[:, 0:1])

            ftg = cpool.tile([128, 64], f16, tag="ftg")
            a16g = cpool.tile([128, 64], f16, tag="a16g")
            ya_0 = cpool.tile([128, FE], f16, tag="ya0")
            ya_1 = cpool.tile([128, FE], f16, tag="ya1")
            ya_2 = cpool.tile([128, FE], f16, tag="ya2")
            ya_3 = cpool.tile([128, FE], f16, tag="ya3")
            yb_0 = cpool.tile([128, FE], f16, tag="yb0")
            yb_1 = cpool.tile([128, FE], f16, tag="yb1")
            yb_2 = cpool.tile([128, FE], f16, tag="yb2")
            yb_3 = cpool.tile([128, FE], f16, tag="yb3")
            ya_t = [ya_0, ya_1, ya_2, ya_3]
            yb_t = [yb_0, yb_1, yb_2, yb_3]

            # ---- both halves: sinc synth + DFT + cmul; iSTFT at the end ----
            s4units = [(i, mt, xdst) for mt, xdst in ((0, xa_sb), (1, xb_sb))
                       for i in range(2)]
            s4units = [s4units[0], s4units[2], s4units[1], s4units[3]]
            fpab0 = cpool.tile([128, 8 * FE], f16, tag="fpab0")
            fpab1 = cpool.tile([128, 8 * FE], f16, tag="fpab1")
            fpabs = [fpab0, fpab1]
            with tc.tile_pool(name="tt5", bufs=2) as ttpool, \
                 tc.tile_pool(name="etp", bufs=2) as etpool2, \
                 tc.tile_pool(name="ftp", bufs=2) as ftpool, \
                 tc.tile_pool(name="fsp", bufs=2) as fspool, \
                 tc.tile_pool(name="a16p", bufs=2) as a16p, \
                 tc.tile_pool(name="zps", bufs=2, space="PSUM") as zps, \
                 tc.tile_pool(name="aps", bufs=2, space="PSUM") as aps:

                def emit_dft(half, oil, fsum, dtl):
                    fpab = fpabs[half]
                    for (mi, m1sb) in ((0, m1a_sb), (1, m1b_sb)):
                        dp = zps.tile([128, 1024], f32, tag="zp")
                        for (f0, nf) in CH:
                            nc.tensor.matmul(dp[:, f0:f0 + nf], m1sb[:],
                                             fsum[:, f0:f0 + nf],
                                             start=True, stop=True)
                        nc.tensor.matmul(dtl[:, mi * 2:mi * 2 + 2], m1sb[:],
                                         fsum[:, 1024:1026],
                                         start=True, stop=True)
                        nc.scalar.activation(
                            fpab[:, (mi * 4 + oil) * FE:(mi * 4 + oil) * FE + 1024],
                            dp[:], AF.Copy)
                    fp_r = fpab[:].rearrange("p (m f) -> p m f", m=2)
                    nc.scalar.activation(
                        fp_r[:, :, oil * FE + 1024:oil * FE + 1026],
                        dtl[:, 0:4].rearrange("p (m t) -> p m t", m=2),
                        AF.Copy)
                def emit_s4(n):
                    for _ in range(n):
                        if not s4units:
                            break
                        i, mt, xdst = s4units.pop(0)
                        x4 = zps.tile([128, 1024], f32, tag="zp")
                        for (f0, nf) in CH:
                            for j in range(4):
                                nc.tensor.matmul(
                                    x4[:, f0:f0 + nf],
                                    wx_sb[:, j * 256 + mt * 128: j * 256 + mt * 128 + 128],
                                    xd_sb[:, i * H2 + f0 + j:i * H2 + f0 + j + nf],
                                    start=(j == 0), stop=(j == 3))
                        nc.scalar.activation(
                            xdst[:, i * FE:i * FE + 1024], x4[:], AF.Copy)

                def emit_stage5(half, ol):
                    fpab = fpabs[half]
                    o = half * 2 + ol
                    ya = ya_t[o]
                    yb = yb_t[o]
                    ta_t = ttpool.tile([128, 2 * FE], f16, tag="tat")
                    nc.vector.tensor_mul(
                        ta_t[:], xa_sb[:],
                        fpab[:, (ol * 2) * FE:(ol * 2 + 2) * FE])
                    nc.vector.tensor_add(ya[:], ta_t[:, 0:FE],
                                         ta_t[:, FE:2 * FE])
                    tb_t = ttpool.tile([128, 2 * FE], f16, tag="tbt")
                    nc.vector.tensor_mul(
                        tb_t[:], xb_sb[:],
                        fpab[:, (4 + ol * 2) * FE:(4 + ol * 2 + 2) * FE])
                    nc.vector.tensor_add(yb[:], tb_t[:, 0:FE],
                                         tb_t[:, FE:2 * FE])
                    ybank[o] = (ya, yb)

                ybank = {}
                pending = None
                emit_s4(4)
                for half in range(2):
                    for oil in range(4):
                        oi = half * 4 + oil
                        ftoi = ftpool.tile([128, 4 * FE], f16, tag="ftoi")
                        ft_r = ftoi[:].rearrange("p (s f) -> p s f", s=4)
                        if oi > 0:
                            nc.gpsimd.tensor_scalar(
                                ft_r[:, :, 1024:1026],
                                ftg[:, oi * 8:oi * 8 + 8],
                                1.0, None, ALU.mult)
                        for si in range(4):
                            ois = oi * 4 + si
                            esl = slice(ois * 128, (ois + 1) * 128)
                            ap = aps.tile([128, 1024], f32, tag="ap")
                            a16 = a16p.tile([128, 1024], f16, tag="a16")
                            zp = zps.tile([128, 1024], f32, tag="zp")
                            for (f0, nf) in CH:
                                nc.tensor.matmul(
                                    zp[:, f0:f0 + nf],
                                    e2z_sb[:, esl], wa_sb[0:33, f0:f0 + nf],
                                    start=True, stop=True)
                            for (f0, nf) in CH:
                                nc.tensor.matmul(
                                    ap[:, f0:f0 + nf],
                                    e2a_sb[:, esl],
                                    wa_sb[0:32, FE + f0:FE + f0 + nf],
                                    start=True, stop=True)
                            nc.scalar.activation(a16[:], ap[:], AF.Copy)
                            nc.vector._custom_dve(
                                sinc_op,
                                out=ftoi[:, si * FE:si * FE + 1024],
                                in0=zp[:], in1=a16[:],
                                s0=SC1, s1=SC2, imm2=SC3)

                        if oi == 0:
                            # global tail pre-pass
                            gt = aps.tile([128, 1024], f32, tag="ap")
                            for ois2 in range(32):
                                es2 = slice(ois2 * 128, (ois2 + 1) * 128)
                                nc.tensor.matmul(
                                    gt[:, ois2 * 2:ois2 * 2 + 2],
                                    e2z_sb[:, es2], wa_sb[0:33, 1024:1026],
                                    start=True, stop=True,
                                    skip_group_check=True)
                                nc.tensor.matmul(
                                    gt[:, 64 + ois2 * 2:64 + ois2 * 2 + 2],
                                    e2a_sb[:, es2],
                                    wa_sb[0:32, FE + 1024:FE + 1026],
                                    start=True, stop=True,
                                    skip_group_check=True)
                            for i in range(2):
                                for mt in range(2):
                                    c0 = 128 + (i * 2 + mt) * 2
                                    for j in range(4):
                                        nc.tensor.matmul(
                                            gt[:, c0:c0 + 2],
                                            wx_sb[:, j * 256 + mt * 128: j * 256 + mt * 128 + 128],
                                            xd_sb[:, i * H2 + 1024 + j:i * H2 + 1026 + j],
                                            start=(j == 0), stop=(j == 3),
                                            skip_group_check=True)
                            nc.scalar.activation(a16g[:], gt[:, 64:128], AF.Copy)
                            nc.vector._custom_dve(
                                sinc_op, out=ftg[:], in0=gt[:, 0:64],
                                in1=a16g[:], s0=SC1, s1=SC2, imm2=SC3)
                            xa_r = xa_sb[:].rearrange("p (i f) -> p i f", i=2)
                            xb_r = xb_sb[:].rearrange("p (i f) -> p i f", i=2)
                            gtr = gt[:, 128:136].rearrange(
                                "p (i mt t) -> p i (mt t)", i=2, mt=2)
                            nc.scalar.activation(
                                xa_r[:, :, 1024:1026], gtr[:, :, 0:2], AF.Copy)
                            nc.scalar.activation(
                                xb_r[:, :, 1024:1026], gtr[:, :, 2:4], AF.Copy)
                            nc.gpsimd.tensor_scalar(
                                ft_r[:, :, 1024:1026], ftg[:, 0:8],
                                1.0, None, ALU.mult)

                        # ---- s-presum ----
                        fs1 = fspool.tile([128, FE], f16, tag="fs1")
                        fsum = fspool.tile([128, FE], f16, tag="fsum")
                        nc.gpsimd.tensor_add(fs1[:], ftoi[:, 0:FE],
                                             ftoi[:, FE:2 * FE])
                        nc.vector.tensor_add(fsum[:], ftoi[:, 2 * FE:3 * FE],
                                             ftoi[:, 3 * FE:4 * FE])
                        nc.vector.tensor_add(fsum[:], fsum[:], fs1[:])
                        dtl = aps.tile([128, 1024], f32, tag="ap")

                        if pending is not None:
                            emit_dft(*pending)
                        pending = (half, oil, fsum, dtl)
                emit_dft(*pending)
                # ---- stage5 + fused iSTFT/OLA + env/bias, interleaved so the
                # DVE cmuls hide under the PE iSTFT stream (yp rides the freed
                # zp rotation) ----
                for o in range(4):
                    emit_stage5(o // 2, o % 2)
                    ya, yb = ybank[o]
                    for c, m0 in ((0, 0), (1, 512)):
                        ypt = zps.tile([128, 1024], f32, tag="zp")
                        yp = ypt[0:64, 0:512]
                        jorder = (0, 1, 2, 3) if c == 0 else (1, 0, 2, 3)
                        first = True
                        k = 0
                        for j in jorder:
                            lo = m0 + 2 - j
                            o0, o1 = 0, 512
                            if lo < 0:
                                o0 = -lo
                            if lo + 512 > F:
                                o1 = F - lo
                            for ic_sb, ysrc in ((ica_sb, ya), (icb_sb, yb)):
                                k += 1
                                nc.tensor.matmul(
                                    yp[:, o0:o1],
                                    ic_sb[:, j * 64:(j + 1) * 64],
                                    ysrc[:, lo + o0:lo + o1],
                                    start=first, stop=(k == 8),
                                    skip_group_check=True)
                                first = False
                        et = etpool2.tile([64, 512], f32, tag="et")
                        nc.vector.tensor_mul(et[:], yp[:],
                                             invt_sb[:, m0:m0 + 512])
                        nc.gpsimd.tensor_scalar(
                            yt_sb[:, o * 1024 + m0:o * 1024 + m0 + 512],
                            et[:], bias_sb[:, o:o + 1], None, ALU.add)
                        nc.sync.dma_start(
                            yt_out[:, o * 1024 + m0:o * 1024 + m0 + 512],
                            yt_sb[:, o * 1024 + m0:o * 1024 + m0 + 512])

    nc.compile()
    return nc


def _prep_inputs(x, conditioning, w1, b1, w2, b2, bias):
    c = _consts()
    x = np.asarray(x, dtype=np.float32)
    conditioning = np.asarray(conditioning, dtype=np.float32)
    w1 = np.asarray(w1, dtype=np.float32)
    b1 = np.asarray(b1, dtype=np.float32)
    w2 = np.asarray(w2, dtype=np.float32)
    b2 = np.asarray(b2, dtype=np.float32)
    bias = np.asarray(bias, dtype=np.float32)

    w1t = w1.reshape(32, 32, 4, 64).transpose(1, 3, 2, 0).reshape(2048, 4, 32)
    w1t_sb = np.ascontiguousarray(
        w1t.reshape(16, 128, 4, 32).transpose(1, 0, 2, 3).reshape(128, 2048)
    ).astype(np.float16)
    w2t = np.ascontiguousarray(w2[:, :, 0].T).astype(np.float16)   # [32, 64]
    bias64 = np.tile(bias.reshape(1, 4), (64, 1)).astype(np.float32)

    shared = {
        "w1t_in": w1t_sb, "w2t_in": w2t,
        "b1_in": b1.reshape(32, 1).astype(np.float32),
        "b2a_in": b2[:32].reshape(32, 1).astype(np.float32),
        "b2w_in": b2[32:].reshape(32, 1).astype(np.float32),
        "e2z_in": c["e2z"], "e2a_in": c["e2a"],
        "m1a_in": c["m1a"], "m1b_in": c["m1b"],
        "wx_in": c["wx_h"], "ica_in": c["ica"], "icb_in": c["icb"],
        "invt_in": c["invt"], "bias_in": bias64, "ones_in": c["ones_row"],
        "id_in": c["id128"],
    }
    L2 = 64 * H2
    in_maps = []
    for b in range(B):
        condpad = np.zeros((CI, L2), dtype=np.float32)
        condpad[:, 128:128 + T] = conditioning[b]
        d = condpad.reshape(CI, H2, 64).transpose(0, 2, 1).reshape(2048, H2)
        d = d.reshape(16, 128, H2)                     # [c, p, col]
        parts = []
        for k in range(8):
            parts.append(d[:, :, k * 128:k * 128 + 132])
        parts.append(d[:, :, 1024:1030])
        d = np.concatenate([p.transpose(1, 0, 2).reshape(128, -1)
                            for p in parts], axis=1)
        d = np.ascontiguousarray(d).astype(np.float16)
        xp = np.zeros((2, L2), dtype=np.float32)
        xp[:, 0:T + 256] = np.pad(x[b], ((0, 0), (128, 128)), mode="reflect")
        xd = np.ascontiguousarray(
            xp.reshape(2, H2, 64).transpose(0, 2, 1).reshape(2, 64, H2)
            .transpose(1, 0, 2).reshape(64, 2 * H2)).astype(np.float16)
        m = dict(shared)
        m["d_in"] = d
        m["xd_in"] = xd
        in_maps.append(m)
    return in_maps


def _assemble(results):
    y = np.empty((B, O, T), dtype=np.float32)
    for b in range(B):
        yt = results[b]["yt_out"]                        # [64, 4096]
        y[b] = yt.reshape(64, 4, 1024).transpose(1, 2, 0).reshape(4, T)
    return y


def kernel(x, conditioning, w1, b1, w2, b2, bias):
    from concourse.bass_utils import run_bass_kernel_spmd
    if "nc" not in _prog_cache:
        _prog_cache["nc"] = _build_program()
    nc = _prog_cache["nc"]
    in_maps = _prep_inputs(x, conditioning, w1, b1, w2, b2, bias)
    res = run_bass_kernel_spmd(nc, in_maps, core_ids=list(range(B)))
    return _assemble(res.results)


# revision 64
# speedup vs baseline: 1.5101x; 1.0154x over previous
"""DynamicSincConv1d Trainium2 kernel (v2).

Data-parallel over batch: 8 batch elements -> 8 NeuronCores. All heavy
math runs on-device in fp16 matmuls (1 cyc/row on the PE) with fp32 PSUM
accumulation; elementwise work is fp16 on the DVE where possible.

Math notes (on top of the v1 symmetry tricks):
 - STFT/irFFT are matmuls against host-baked DFT matrices with windows
   folded in; the sinc bank is symmetric about k=128 so only d=|k-128|
   in [0,128) is materialized and every filter spectrum is real.
 - z = ta_d*wid + pi*eps is produced directly by the broadcast matmul:
   the one-hot selector is pre-scaled by ta_d and an appended ones-row
   carries the +pi*eps bias.
 - sinc(z) = P(z^2) (degree-3 minimax fit, |err| < 1e-6 over the full
   z range |z| <= pi/2) evaluated together with the amp multiply in a
   single custom DVE op: out = (u*(c1 + u*(c2 + u*c3)))*amp + amp.
 - The 4 sinc banks are pre-summed on the DVE so the filter DFT is a
   single 128x128 matmul stream per (oi, a/b) tile.
 - Overlap-add is folded into the iSTFT matmul accumulation: the four
   frame-shifted contributions accumulate in PSUM via column-shifted
   moving operands, then one env-normalize + bias pass finishes y.
 - F is padded 1025 -> 1026; the junk frame never reaches the output
   because the fused-OLA matmuls exclude it by construction.
"""

import math
import numpy as np

B, CI, I, O, S = 8, 32, 2, 4, 4
K, HOP, T = 256, 64, 65536
F = T // HOP + 1             # 1025 real frames
FE = F + 1                   # padded frame count (col 1025 is junk)
H2 = (T + K) // HOP + 2      # 1030 padded hop columns (frame f uses cols f..f+3)
PI = math.pi
PIEPS = PI * 1e-6
# degree-3 minimax fit of sinc(z) = 1 + u*(c1 + u*(c2 + u*c3)), u = z^2
SC1, SC2, SC3 = -0.16665935405036883, 0.008315297713095644, -0.00018570764930803712
M1S = 16.0                   # m1 scale (fp16 range); folded out via invt
ICS = 16.0                   # ic scale; folded out via invt
CH = [(0, 512), (512, 512)]  # main column chunks; tail cols [1024:1026]

_prog_cache = {}


def _register_sinc_amp():
    import concourse.dve_ops as dve_ops
    from concourse.dve_spec import Spec, Src0, Src1, C0, C1, C2, sq, lower
    from concourse.dve_uop import DveOpSpec

    for op in dve_ops.OPS:
        if op.name == "SINC_AMP_ANT":
            return op
    u = sq(Src0)
    body = (u * (C0 + u * (C1 + u * C2))) * Src1 + Src1

    def ref(in0, in1, s0, s1, imm2):
        uu = np.square(in0.astype(np.float32))
        return ((uu * (s0 + uu * (s1 + uu * imm2))) * in1 + in1).astype(np.float32)

    spec = Spec(body=body, reference=ref)
    row = max(dve_ops._SUB_OPCODE_FOR_NAME.values()) + 1
    assert row < 0x20
    shas = {}
    for ver in ("v3", "v4"):
        uops = lower(spec, ver=ver)
        shas[ver] = DveOpSpec(
            name="SINC_AMP_ANT", opcode=row, uops=uops, rd1_en=True
        ).sha(ver)
    op = dve_ops.DveOp("SINC_AMP_ANT", spec, subdim=False, uops_sha=shas)
    dve_ops.OPS.append(op)
    dve_ops.CUSTOM_DVE_SPECS["SINC_AMP_ANT"] = spec
    dve_ops._SUB_OPCODE_FOR_NAME["SINC_AMP_ANT"] = row
    return op


def _consts():
    n = np.arange(K, dtype=np.float64)
    ola = 0.5 * (1.0 - np.cos(2.0 * np.pi * n / K))
    fir = 0.42 - 0.5 * np.cos(2.0 * np.pi * n / K) + 0.08 * np.cos(4.0 * np.pi * n / K)

    d = np.arange(128, dtype=np.float64)
    ta = (PI * d / K)                                  # pi*d/256

    # M1[d, fb] = (-1)^fb * c_d * fir[128+d]/(S*K) * cos(2*pi*d*fb/K), x M1S
    fb = np.arange(K // 2 + 1, dtype=np.float64)       # 0..128
    cd = np.where(d == 0, 1.0, 2.0)
    m1 = (((-1.0) ** fb)[None, :] * cd[:, None] * fir[128 + d.astype(int)][:, None]
          / (S * K) * np.cos(2.0 * np.pi * np.outer(d, fb) / K)) * M1S
    m1a = np.ascontiguousarray(m1[:, 0:128]).astype(np.float16)
    m1b = np.concatenate([m1[:, 128:129], m1[:, 1:128]], axis=1).astype(np.float16)

    # STFT weights: wx[j][r, col]; k = 64 j + r; fbpack col layout
    kk = np.arange(K, dtype=np.float64)
    ang = 2.0 * np.pi * np.outer(kk, fb) / K           # [256, 129]
    wre = ola[:, None] * np.cos(ang)
    wim = -ola[:, None] * np.sin(ang)
    colsA = wre[:, 0:128]
    colsB = np.concatenate([wre[:, 128:129], wim[:, 1:128]], axis=1)
    wx_full = np.concatenate([colsA, colsB], axis=1)   # [256, 256]
    wx = wx_full.reshape(4, 64, 256)
    wx_h = np.ascontiguousarray(
        wx.transpose(1, 0, 2).reshape(64, 1024)).astype(np.float16)

    # iSTFT: IC[fbpack_row, n] with ola folded, x ICS
    cp = np.where(fb == 0, 1.0, 2.0)
    icre = (cp[:, None] / K) * np.cos(2.0 * np.pi * np.outer(fb, n) / K) * ola[None, :] * ICS
    icim = (-2.0 / K) * np.sin(2.0 * np.pi * np.outer(fb, n) / K) * ola[None, :] * ICS
    ica = np.ascontiguousarray(icre[0:128]).astype(np.float16)                    # [128, 256]
    icb = np.concatenate([icre[128:129], icim[1:128]], axis=0).astype(np.float16)  # [128, 256]

    # env inverse, arranged [r, p]; absorb 1/(M1S*ICS)
    ola2 = ola * ola
    env_q = np.zeros((1028, 64), dtype=np.float64)
    for j in range(4):
        env_q[j:F + j, :] += ola2[64 * j:64 * j + 64][None, :]
    invt = (1.0 / (env_q[2:2 + 1024, :] * (M1S * ICS))).T.astype(np.float32)
    invt = np.ascontiguousarray(invt)                  # [64, 1024]

    # z-broadcast selector [33, 4096]: rows q<32: (q==ois)*ta_d; row 32: pieps
    ta16 = ta.astype(np.float16)
    e2z = np.zeros((33, 32 * 128), dtype=np.float16)
    e2a = np.zeros((32, 32 * 128), dtype=np.float16)
    for q in range(32):
        e2z[q, q * 128:(q + 1) * 128] = ta16
        e2a[q, q * 128:(q + 1) * 128] = 1.0
    e2z[32, :] = np.float16(PIEPS)
    ones_row = np.ones((1, FE), dtype=np.float16)
    id128 = np.eye(128, dtype=np.float16)
    return dict(id128=id128, m1a=m1a, m1b=m1b, wx_h=wx_h, ica=ica, icb=icb, invt=invt,
                e2z=e2z, e2a=e2a, ones_row=ones_row)


def _build_program():
    import concourse.bacc as bacc
    import concourse.mybir as mybir
    import concourse.tile as tile

    sinc_op = _register_sinc_amp()

    f16 = mybir.dt.float16
    f32 = mybir.dt.float32
    AF = mybir.ActivationFunctionType
    ALU = mybir.AluOpType

    nc = bacc.Bacc("TRN2", target_bir_lowering=False, debug=False, num_devices=8)

    d_in = nc.dram_tensor("d_in", [128, 8 * 2112 + 96], f16, kind="ExternalInput")
    xd_in = nc.dram_tensor("xd_in", [64, 2 * H2], f16, kind="ExternalInput")
    w1t_in = nc.dram_tensor("w1t_in", [128, 2048], f16, kind="ExternalInput")
    w2t_in = nc.dram_tensor("w2t_in", [32, 64], f16, kind="ExternalInput")
    b1_in = nc.dram_tensor("b1_in", [32, 1], f32, kind="ExternalInput")
    b2a_in = nc.dram_tensor("b2a_in", [32, 1], f32, kind="ExternalInput")
    b2w_in = nc.dram_tensor("b2w_in", [32, 1], f32, kind="ExternalInput")
    e2z_in = nc.dram_tensor("e2z_in", [33, 4096], f16, kind="ExternalInput")
    e2a_in = nc.dram_tensor("e2a_in", [32, 4096], f16, kind="ExternalInput")
    m1a_in = nc.dram_tensor("m1a_in", [128, 128], f16, kind="ExternalInput")
    m1b_in = nc.dram_tensor("m1b_in", [128, 128], f16, kind="ExternalInput")
    wx_in = nc.dram_tensor("wx_in", [64, 1024], f16, kind="ExternalInput")
    ica_in = nc.dram_tensor("ica_in", [128, 256], f16, kind="ExternalInput")
    icb_in = nc.dram_tensor("icb_in", [128, 256], f16, kind="ExternalInput")
    invt_in = nc.dram_tensor("invt_in", [64, 1024], f32, kind="ExternalInput")
    bias_in = nc.dram_tensor("bias_in", [64, 4], f32, kind="ExternalInput")
    ones_in = nc.dram_tensor("ones_in", [1, FE], f16, kind="ExternalInput")
    id_in = nc.dram_tensor("id_in", [128, 128], f16, kind="ExternalInput")
    yt_out = nc.dram_tensor("yt_out", [64, 4096], f32, kind="ExternalOutput")

    with tile.TileContext(nc) as tc:
        with tc.tile_pool(name="cpool", bufs=1) as cpool:
            w1t_sb = cpool.tile([128, 2048], f16, tag="w1t")
            w2t_sb = cpool.tile([32, 64], f16, tag="w2t")
            b1_sb = cpool.tile([32, 1], f32, tag="b1")
            b2a_sb = cpool.tile([32, 1], f32, tag="b2a")
            b2w_sb = cpool.tile([32, 1], f32, tag="b2w")
            e2z_sb = cpool.tile([33, 4096], f16, tag="e2z")
            e2a_sb = cpool.tile([32, 4096], f16, tag="e2a")
            m1a_sb = cpool.tile([128, 128], f16, tag="m1a")
            m1b_sb = cpool.tile([128, 128], f16, tag="m1b")
            wx_sb = cpool.tile([64, 1024], f16, tag="wx")
            ica_sb = cpool.tile([128, 256], f16, tag="ica")
            icb_sb = cpool.tile([128, 256], f16, tag="icb")
            invt_sb = cpool.tile([64, 1024], f32, tag="invt")
            bias_sb = cpool.tile([64, 4], f32, tag="bias")
            xd_sb = cpool.tile([64, 2 * H2], f16, tag="xd")
            h_sb = cpool.tile([32, FE], f16, tag="h")
            wa_sb = cpool.tile([33, 2 * FE], f16, tag="wa")
            xa_sb = cpool.tile([128, 2 * FE], f16, tag="xa")
            xb_sb = cpool.tile([128, 2 * FE], f16, tag="xb")
            yt_sb = cpool.tile([64, 4096], f32, tag="yt")
            id_sb = cpool.tile([128, 128], f16, tag="id128")

            # load order matters: stage-1 operands first, spread over the
            # two HWDGE queues (SP + Activation)
            nc.sync.dma_start(w1t_sb[:], w1t_in[:])
            nc.scalar.dma_start(b1_sb[:], b1_in[:])
            nc.scalar.dma_start(id_sb[:], id_in[:])
            nc.scalar.dma_start(wa_sb[32:33, 0:FE], ones_in[:])

            # ---- stage 1: conditioning conv -> h [32, FE] fp16 ----
            with tc.tile_pool(name="dpool", bufs=1) as dpool, \
                 tc.tile_pool(name="hts", bufs=2) as htsp, \
                 tc.tile_pool(name="ps1", bufs=2, space="PSUM") as ps1, \
                 tc.tile_pool(name="ps2", bufs=2, space="PSUM") as ps2, \
                 tc.tile_pool(name="psT", bufs=2, space="PSUM") as psT:
                # band-major d: band k holds cols [k*128, k*128+132) of all
                # 16 c-chunks contiguously; ftile k's matmuls start as soon as
                # band k lands.
                dbig = dpool.tile([128, 8 * 2112 + 96], f16, tag="dbig")
                BW = [2112] * 8 + [96]
                boff = [0]
                for w in BW:
                    boff.append(boff[-1] + w)
                for k in range(9):
                    nc.sync.dma_start(dbig[:, boff[k]:boff[k + 1]],
                                      d_in[:, boff[k]:boff[k + 1]])
                for t_sb, t_in in (
                        (w2t_sb, w2t_in), (b2w_sb, b2w_in), (b2a_sb, b2a_in),
                        (xd_sb, xd_in), (e2z_sb, e2z_in), (e2a_sb, e2a_in),
                        (wx_sb, wx_in),
                        (m1a_sb, m1a_in), (m1b_sb, m1b_in),
                        (ica_sb, ica_in), (icb_sb, icb_in), (invt_sb, invt_in),
                        (bias_sb, bias_in)):
                    nc.sync.dma_start(t_sb[:], t_in[:])
                FT1 = [(k, 128) for k in range(0, 1024, 128)] + [(1024, 2)]
                for ft, (f0, nf) in enumerate(FT1):
                    cw = 132 if ft < 8 else 6
                    ps = ps1.tile([128, 32], f32, tag="ps1")
                    idx = 0
                    for c in range(16):
                        for j in range(4):
                            nc.tensor.matmul(
                                ps[0:nf, :],
                                dbig[:, boff[ft] + c * cw + j:
                                     boff[ft] + c * cw + j + nf],
                                w1t_sb[:, (c * 4 + j) * 32:(c * 4 + j + 1) * 32],
                                start=(idx == 0), stop=(idx == 63))
                            idx += 1
                    ht16 = htsp.tile([128, 32], f16, tag="ht16")
                    nc.scalar.activation(ht16[0:nf, :], ps[0:nf, :], AF.Copy)
                    pt = psT.tile([32, 128], f16, tag="psT")
                    nc.tensor.matmul(pt[:, 0:nf], ht16[0:nf, :],
                                     id_sb[0:nf, 0:nf],
                                     start=True, stop=True, is_transpose=True)
                    nc.scalar.activation(h_sb[:, f0:f0 + nf], pt[:, 0:nf],
                                         AF.Identity, bias=b1_sb[:, 0:1])
                    # leaky-relu + stage 2 chunkwise as soon as h cols land
                    if ft in (3, 7, 8):
                        c0 = {3: 0, 7: 512, 8: 1024}[ft]
                        nc_ = {3: 512, 7: 512, 8: 2}[ft]
                        lt = dpool.tile([32, 512], f16, tag="lt")
                        nc.vector.tensor_scalar(lt[:, 0:nc_], h_sb[:, c0:c0 + nc_],
                                                0.01, None, ALU.mult)
                        nc.vector.tensor_max(h_sb[:, c0:c0 + nc_],
                                             h_sb[:, c0:c0 + nc_], lt[:, 0:nc_])
                        pw = ps2.tile([32, nc_ if nc_ > 2 else 2], f32, tag="ps2w")
                        nc.tensor.matmul(pw[:], w2t_sb[:, 32:64],
                                         h_sb[:, c0:c0 + nc_],
                                         start=True, stop=True)
                        nc.scalar.activation(wa_sb[0:32, c0:c0 + nc_], pw[:],
                                             AF.Tanh, bias=b2w_sb[:, 0:1])
                        pa = ps2.tile([32, nc_ if nc_ > 2 else 2], f32, tag="ps2a")
                        nc.tensor.matmul(pa[:], w2t_sb[:, 0:32],
                                         h_sb[:, c0:c0 + nc_],
                                         start=True, stop=True)
                        nc.scalar.activation(wa_sb[0:32, FE + c0:FE + c0 + nc_],
                                             pa[:], AF.Tanh, bias=b2a_sb[:, 0:1])

            ftg = cpool.tile([128, 64], f16, tag="ftg")
            a16g = cpool.tile([128, 64], f16, tag="a16g")
            ampP0 = cpool.tile([1, 16 * 1024], f16, tag="ampP0")
            ya_0 = cpool.tile([128, FE], f16, tag="ya0")
            ya_1 = cpool.tile([128, FE], f16, tag="ya1")
            ya_2 = cpool.tile([128, FE], f16, tag="ya2")
            ya_3 = cpool.tile([128, FE], f16, tag="ya3")
            yb_0 = cpool.tile([128, FE], f16, tag="yb0")
            yb_1 = cpool.tile([128, FE], f16, tag="yb1")
            yb_2 = cpool.tile([128, FE], f16, tag="yb2")
            yb_3 = cpool.tile([128, FE], f16, tag="yb3")
            ya_t = [ya_0, ya_1, ya_2, ya_3]
            yb_t = [yb_0, yb_1, yb_2, yb_3]

            # ---- both halves: sinc synth + DFT + cmul; iSTFT at the end ----
            s4units = [(i, mt, xdst) for mt, xdst in ((0, xa_sb), (1, xb_sb))
                       for i in range(2)]
            s4units = [s4units[0], s4units[2], s4units[1], s4units[3]]
            # stage the s=2,3 amp rows at partition 0 (SBUF->SBUF DMA on the
            # idle post-load queue) so Pool partition_broadcast can legally
            # replicate them, skipping the PE matmul + psum + Act copy.
            for oi_ in range(8):
                for si_ in (2, 3):
                    k_ = oi_ * 2 + (si_ - 2)
                    nc.sync.dma_start(
                        ampP0[0:1, k_ * 1024:(k_ + 1) * 1024],
                        wa_sb[oi_ * 4 + si_:oi_ * 4 + si_ + 1, FE:FE + 1024])
            fpab0 = cpool.tile([128, 8 * FE], f16, tag="fpab0")
            fpab1 = cpool.tile([128, 8 * FE], f16, tag="fpab1")
            fpabs = [fpab0, fpab1]
            with tc.tile_pool(name="tt5", bufs=2) as ttpool, \
                 tc.tile_pool(name="etp", bufs=2) as etpool2, \
                 tc.tile_pool(name="ftp", bufs=2) as ftpool, \
                 tc.tile_pool(name="fsp", bufs=2) as fspool, \
                 tc.tile_pool(name="a16p", bufs=4) as a16p, \
                 tc.tile_pool(name="zps", bufs=2, space="PSUM") as zps, \
                 tc.tile_pool(name="aps", bufs=2, space="PSUM") as aps:

                def emit_dft(half, oil, fsum, dtl):
                    fpab = fpabs[half]
                    for (mi, m1sb) in ((0, m1a_sb), (1, m1b_sb)):
                        dp = zps.tile([128, 1024], f32, tag="zp")
                        for (f0, nf) in CH:
                            nc.tensor.matmul(dp[:, f0:f0 + nf], m1sb[:],
                                             fsum[:, f0:f0 + nf],
                                             start=True, stop=True)
                        nc.tensor.matmul(dtl[:, mi * 2:mi * 2 + 2], m1sb[:],
                                         fsum[:, 1024:1026],
                                         start=True, stop=True)
                        nc.scalar.activation(
                            fpab[:, (mi * 4 + oil) * FE:(mi * 4 + oil) * FE + 1024],
                            dp[:], AF.Copy)
                    fp_r = fpab[:].rearrange("p (m f) -> p m f", m=2)
                    nc.scalar.activation(
                        fp_r[:, :, oil * FE + 1024:oil * FE + 1026],
                        dtl[:, 0:4].rearrange("p (m t) -> p m t", m=2),
                        AF.Copy)
                def emit_s4(n):
                    for _ in range(n):
                        if not s4units:
                            break
                        i, mt, xdst = s4units.pop(0)
                        x4 = zps.tile([128, 1024], f32, tag="zp")
                        for (f0, nf) in CH:
                            for j in range(4):
                                nc.tensor.matmul(
                                    x4[:, f0:f0 + nf],
                                    wx_sb[:, j * 256 + mt * 128: j * 256 + mt * 128 + 128],
                                    xd_sb[:, i * H2 + f0 + j:i * H2 + f0 + j + nf],
                                    start=(j == 0), stop=(j == 3))
                        nc.scalar.activation(
                            xdst[:, i * FE:i * FE + 1024], x4[:], AF.Copy)

                def emit_stage5(half, ol):
                    fpab = fpabs[half]
                    o = half * 2 + ol
                    ya = ya_t[o]
                    yb = yb_t[o]
                    ta_t = ttpool.tile([128, 2 * FE], f16, tag="tat")
                    nc.vector.tensor_mul(
                        ta_t[:], xa_sb[:],
                        fpab[:, (ol * 2) * FE:(ol * 2 + 2) * FE])
                    nc.vector.tensor_add(ya[:], ta_t[:, 0:FE],
                                         ta_t[:, FE:2 * FE])
                    tb_t = ttpool.tile([128, 2 * FE], f16, tag="tbt")
                    nc.vector.tensor_mul(
                        tb_t[:], xb_sb[:],
                        fpab[:, (4 + ol * 2) * FE:(4 + ol * 2 + 2) * FE])
                    nc.vector.tensor_add(yb[:], tb_t[:, 0:FE],
                                         tb_t[:, FE:2 * FE])
                    ybank[o] = (ya, yb)

                ybank = {}
                pending = None
                emit_s4(4)
                for half in range(2):
                    for oil in range(4):
                        oi = half * 4 + oil
                        ftoi = ftpool.tile([128, 4 * FE], f16, tag="ftoi")
                        ft_r = ftoi[:].rearrange("p (s f) -> p s f", s=4)
                        if oi > 0:
                            nc.gpsimd.tensor_scalar(
                                ft_r[:, :, 1024:1026],
                                ftg[:, oi * 8:oi * 8 + 8],
                                1.0, None, ALU.mult)
                        a16_pb = {}
                        for si in (2, 3):
                            k_ = oi * 2 + (si - 2)
                            a16 = a16p.tile([128, 1024], f16, tag="a16")
                            nc.gpsimd.partition_broadcast(
                                a16[:], ampP0[0:1, k_ * 1024:(k_ + 1) * 1024])
                            a16_pb[si] = a16
                        for si in range(4):
                            ois = oi * 4 + si
                            esl = slice(ois * 128, (ois + 1) * 128)
                            zp = zps.tile([128, 1024], f32, tag="zp")
                            for (f0, nf) in CH:
                                nc.tensor.matmul(
                                    zp[:, f0:f0 + nf],
                                    e2z_sb[:, esl], wa_sb[0:33, f0:f0 + nf],
                                    start=True, stop=True)
                            if si < 2:
                                ap = aps.tile([128, 1024], f32, tag="ap")
                                a16 = a16p.tile([128, 1024], f16, tag="a16")
                                for (f0, nf) in CH:
                                    nc.tensor.matmul(
                                        ap[:, f0:f0 + nf],
                                        e2a_sb[:, esl],
                                        wa_sb[0:32, FE + f0:FE + f0 + nf],
                                        start=True, stop=True)
                                nc.scalar.activation(a16[:], ap[:], AF.Copy)
                            else:
                                a16 = a16_pb[si]
                            nc.vector._custom_dve(
                                sinc_op,
                                out=ftoi[:, si * FE:si * FE + 1024],
                                in0=zp[:], in1=a16[:],
                                s0=SC1, s1=SC2, imm2=SC3)

                        if oi == 0:
                            # global tail pre-pass
                            gt = aps.tile([128, 1024], f32, tag="ap")
                            for ois2 in range(32):
                                es2 = slice(ois2 * 128, (ois2 + 1) * 128)
                                nc.tensor.matmul(
                                    gt[:, ois2 * 2:ois2 * 2 + 2],
                                    e2z_sb[:, es2], wa_sb[0:33, 1024:1026],
                                    start=True, stop=True,
                                    skip_group_check=True)
                                nc.tensor.matmul(
                                    gt[:, 64 + ois2 * 2:64 + ois2 * 2 + 2],
                                    e2a_sb[:, es2],
                                    wa_sb[0:32, FE + 1024:FE + 1026],
                                    start=True, stop=True,
                                    skip_group_check=True)
                            for i in range(2):
                                for mt in range(2):
                                    c0 = 128 + (i * 2 + mt) * 2
                                    for j in range(4):
                                        nc.tensor.matmul(
                                            gt[:, c0:c0 + 2],
                                            wx_sb[:, j * 256 + mt * 128: j * 256 + mt * 128 + 128],
                                            xd_sb[:, i * H2 + 1024 + j:i * H2 + 1026 + j],
                                            start=(j == 0), stop=(j == 3),
                                            skip_group_check=True)
                            nc.scalar.activation(a16g[:], gt[:, 64:128], AF.Copy)
                            nc.vector._custom_dve(
                                sinc_op, out=ftg[:], in0=gt[:, 0:64],
                                in1=a16g[:], s0=SC1, s1=SC2, imm2=SC3)
                            xa_r = xa_sb[:].rearrange("p (i f) -> p i f", i=2)
                            xb_r = xb_sb[:].rearrange("p (i f) -> p i f", i=2)
                            gtr = gt[:, 128:136].rearrange(
                                "p (i mt t) -> p i (mt t)", i=2, mt=2)
                            nc.scalar.activation(
                                xa_r[:, :, 1024:1026], gtr[:, :, 0:2], AF.Copy)
                            nc.scalar.activation(
                                xb_r[:, :, 1024:1026], gtr[:, :, 2:4], AF.Copy)
                            nc.gpsimd.tensor_scalar(
                                ft_r[:, :, 1024:1026], ftg[:, 0:8],
                                1.0, None, ALU.mult)

                        # ---- s-presum ----
                        fs1 = fspool.tile([128, FE], f16, tag="fs1")
                        fsum = fspool.tile([128, FE], f16, tag="fsum")
                        nc.gpsimd.tensor_add(fs1[:], ftoi[:, 0:FE],
                                             ftoi[:, FE:2 * FE])
                        nc.vector.tensor_add(fsum[:], ftoi[:, 2 * FE:3 * FE],
                                             ftoi[:, 3 * FE:4 * FE])
                        nc.vector.tensor_add(fsum[:], fsum[:], fs1[:])
                        dtl = aps.tile([128, 1024], f32, tag="ap")

                        if pending is not None:
                            emit_dft(*pending)
                        pending = (half, oil, fsum, dtl)
                emit_dft(*pending)
                # ---- stage5 + fused iSTFT/OLA + env/bias, interleaved so the
                # DVE cmuls hide under the PE iSTFT stream (yp rides the freed
                # zp rotation) ----
                for o in range(4):
                    emit_stage5(o // 2, o % 2)
                    ya, yb = ybank[o]
                    for c, m0 in ((0, 0), (1, 512)):
                        ypt = zps.tile([128, 1024], f32, tag="zp")
                        yp = ypt[0:64, 0:512]
                        jorder = (0, 1, 2, 3) if c == 0 else (1, 0, 2, 3)
                        first = True
                        k = 0
                        for j in jorder:
                            lo = m0 + 2 - j
                            o0, o1 = 0, 512
                            if lo < 0:
                                o0 = -lo
                            if lo + 512 > F:
                                o1 = F - lo
                            for ic_sb, ysrc in ((ica_sb, ya), (icb_sb, yb)):
                                k += 1
                                nc.tensor.matmul(
                                    yp[:, o0:o1],
                                    ic_sb[:, j * 64:(j + 1) * 64],
                                    ysrc[:, lo + o0:lo + o1],
                                    start=first, stop=(k == 8),
                                    skip_group_check=True)
                                first = False
                        et = etpool2.tile([64, 512], f32, tag="et")
                        nc.vector.tensor_mul(et[:], yp[:],
                                             invt_sb[:, m0:m0 + 512])
                        nc.gpsimd.tensor_scalar(
                            yt_sb[:, o * 1024 + m0:o * 1024 + m0 + 512],
                            et[:], bias_sb[:, o:o + 1], None, ALU.add)
                        nc.sync.dma_start(
                            yt_out[:, o * 1024 + m0:o * 1024 + m0 + 512],
                            yt_sb[:, o * 1024 + m0:o * 1024 + m0 + 512])

    nc.compile()
    return nc


def _prep_inputs(x, conditioning, w1, b1, w2, b2, bias):
    c = _consts()
    x = np.asarray(x, dtype=np.float32)
    conditioning = np.asarray(conditioning, dtype=np.float32)
    w1 = np.asarray(w1, dtype=np.float32)
    b1 = np.asarray(b1, dtype=np.float32)
    w2 = np.asarray(w2, dtype=np.float32)
    b2 = np.asarray(b2, dtype=np.float32)
    bias = np.asarray(bias, dtype=np.float32)

    w1t = w1.reshape(32, 32, 4, 64).transpose(1, 3, 2, 0).reshape(2048, 4, 32)
    w1t_sb = np.ascontiguousarray(
        w1t.reshape(16, 128, 4, 32).transpose(1, 0, 2, 3).reshape(128, 2048)
    ).astype(np.float16)
    w2t = np.ascontiguousarray(w2[:, :, 0].T).astype(np.float16)   # [32, 64]
    bias64 = np.tile(bias.reshape(1, 4), (64, 1)).astype(np.float32)

    shared = {
        "w1t_in": w1t_sb, "w2t_in": w2t,
        "b1_in": b1.reshape(32, 1).astype(np.float32),
        "b2a_in": b2[:32].reshape(32, 1).astype(np.float32),
        "b2w_in": b2[32:].reshape(32, 1).astype(np.float32),
        "e2z_in": c["e2z"], "e2a_in": c["e2a"],
        "m1a_in": c["m1a"], "m1b_in": c["m1b"],
        "wx_in": c["wx_h"], "ica_in": c["ica"], "icb_in": c["icb"],
        "invt_in": c["invt"], "bias_in": bias64, "ones_in": c["ones_row"],
        "id_in": c["id128"],
    }
    L2 = 64 * H2
    in_maps = []
    for b in range(B):
        condpad = np.zeros((CI, L2), dtype=np.float32)
        condpad[:, 128:128 + T] = conditioning[b]
        d = condpad.reshape(CI, H2, 64).transpose(0, 2, 1).reshape(2048, H2)
        d = d.reshape(16, 128, H2)                     # [c, p, col]
        parts = []
        for k in range(8):
            parts.append(d[:, :, k * 128:k * 128 + 132])
        parts.append(d[:, :, 1024:1030])
        d = np.concatenate([p.transpose(1, 0, 2).reshape(128, -1)
                            for p in parts], axis=1)
        d = np.ascontiguousarray(d).astype(np.float16)
        xp = np.zeros((2, L2), dtype=np.float32)
        xp[:, 0:T + 256] = np.pad(x[b], ((0, 0), (128, 128)), mode="reflect")
        xd = np.ascontiguousarray(
            xp.reshape(2, H2, 64).transpose(0, 2, 1).reshape(2, 64, H2)
            .transpose(1, 0, 2).reshape(64, 2 * H2)).astype(np.float16)
        m = dict(shared)
        m["d_in"] = d
        m["xd_in"] = xd
        in_maps.append(m)
    return in_maps


def _assemble(results):
    y = np.empty((B, O, T), dtype=np.float32)
    for b in range(B):
        yt = results[b]["yt_out"]                        # [64, 4096]
        y[b] = yt.reshape(64, 4, 1024).transpose(1, 2, 0).reshape(4, T)
    return y


def kernel(x, conditioning, w1, b1, w2, b2, bias):
    from concourse.bass_utils import run_bass_kernel_spmd
    if "nc" not in _prog_cache:
        _prog_cache["nc"] = _build_program()
    nc = _prog_cache["nc"]
    in_maps = _prep_inputs(x, conditioning, w1, b1, w2, b2, bias)
    res = run_bass_kernel_spmd(nc, in_maps, core_ids=list(range(B)))
    return _assemble(res.results)
